# revision 15
# baseline (speedup 1.0000x reference)
"""Trainium2 Bass kernel for nn_MultiHeadLayer (pre-LN MHA, fused QKV).

Self-contained: takes FULL inputs, shards data-parallel over batch across
8 NeuronCores, runs a Bass/Tile kernel per core, gathers the full output.

Per-core dataflow (T = B_core*S tokens, H hidden, NH heads, D = H/NH):
  Phase 1: host supplies xT [H, T] bf16 (pre-transposed). LN without PE
           transposes: Sx = 1s @ xT and Sxx = 1s @ xT^2 ones-matmuls land
           the per-token mean/var broadcast across partitions in PSUM for
           free. The mean is NOT subtracted from x; instead the identity
           (x - mu) @ W = x @ W - mu * colsum(W) is applied at the PSUM
           evacuation of the QKV matmul together with the 1/std factor
           (colsum(W) precomputed on host). All-bf16 weight-stationary
           matmuls. q,k stream to projT [2H, T] bf16 in DRAM; v is
           PE-transposed at evacuation time into an SBUF-resident
           vn_all [tok, d] so phase 2 needs no v traffic at all.
  Phase 2: per (batch, head): scoresT = kT.T @ qT (k on partitions), exp
           fused with additive mask via per-partition ACT bias, sumexp via
           ones-matmul, ctxT from vn_all with the normalization fused into
           the PSUM evacuation. q,k are loaded per head-pair x batch-half
           group as single large row-contiguous DMAs on the sync queue;
           the scalar queue carries only exp + o-weight prefetch.
  Phase 3: outT = o.T @ ctxT (bf16), weight-stationary -> outT [H, T] ->
           host transposes during unshard.
"""

import numpy as np
from functools import lru_cache

LN_EPS = 1e-5
NEG_BIG = -1.0e30


def _build(n_cores, T, S, H, NH, is_pre, has_bias, repeat=1):
    import concourse.bacc as bacc
    import concourse.mybir as mybir
    import concourse.tile as tile
    from concourse.masks import make_identity

    F32 = mybir.dt.float32
    BF16 = mybir.dt.bfloat16
    ACT = mybir.ActivationFunctionType

    KO = H // 128          # hidden-dim 128-chunks
    H3 = 3 * H
    D = H // NH
    DT = D // 128          # d-chunks per head
    KT = S // 128          # key-token 128-chunks per sequence
    B_core = T // S
    TC = T // 512          # token 512-chunks
    NCH = H3 // 128        # qkv column chunks of 128
    NQK = 2 * H // 128     # q+k column chunks (written to DRAM)

    nc = bacc.Bacc("TRN2", target_bir_lowering=False, debug=False,
                   num_devices=n_cores)

    xT_d = nc.dram_tensor("xT", [KO, 128, T], BF16, kind="ExternalInput")
    qkv_d = nc.dram_tensor("qkvw", [KO, 128, H3], BF16, kind="ExternalInput")
    o_d = nc.dram_tensor("ow", [KO, 128, H], BF16, kind="ExternalInput")
    # maskb[b*KT+kt, :] = additive key-mask bias for key tokens kt*128..+128
    mb_d = nc.dram_tensor("maskb", [B_core * KT, 128], F32,
                          kind="ExternalInput")
    if is_pre:
        # csum[i, :] = sum_h qkvw_bf16[h, i*128:(i+1)*128] (host, exact)
        cs_d = nc.dram_tensor("csum", [NCH, 128], F32, kind="ExternalInput")
    if has_bias:
        # bqkv[i, :] = (bias @ qkvw)[i*128:(i+1)*128]
        bq_d = nc.dram_tensor("bqkv", [NCH, 128], F32, kind="ExternalInput")
    if is_pre:
        out_d = nc.dram_tensor("outT", [H, T], F32, kind="ExternalOutput")
    else:
        # post-LN needs LN params applied on-device to the output rows
        lnw_d = nc.dram_tensor("lnw", [H], F32, kind="ExternalInput")
        lnb_d = nc.dram_tensor("lnb", [H], F32, kind="ExternalInput")
        out_d = nc.dram_tensor("outN", [T, H], F32, kind="ExternalOutput")

    with tile.TileContext(nc) as tc:
        with tc.tile_pool(name="consts", bufs=1) as cp, \
             tc.tile_pool(name="dram", bufs=1, space="DRAM") as dp:
            ident = cp.tile([128, 128], F32)
            make_identity(nc, ident[:])
            identb = cp.tile([128, 128], BF16)
            nc.vector.tensor_copy(identb[:], ident[:])
            onesb = cp.tile([128, 128], BF16)
            nc.vector.memset(onesb[:], 1.0)
            eps_t = cp.tile([128, 1], F32)
            nc.vector.memset(eps_t[:], LN_EPS)
            mb_t = cp.tile([128, B_core * KT], F32)
            nc.sync.dma_start(mb_t[:], mb_d.ap().rearrange("i p -> p i"))
            if is_pre:
                cs_t = cp.tile([128, NCH], F32)
                nc.sync.dma_start(cs_t[:], cs_d.ap().rearrange("i p -> p i"))
            if has_bias:
                bq_t = cp.tile([128, NCH], F32)
                nc.sync.dma_start(bq_t[:], bq_d.ap().rearrange("i p -> p i"))

            qkv_ap = qkv_d.ap().rearrange("ko p n -> p ko n")
            o_ap = o_d.ap().rearrange("ko p n -> p ko n")
            projT = dp.tile([2 * H, T], BF16)
            if not is_pre:
                oTs = dp.tile([H, T], F32)
                import concourse.bass as _bass
                lnw_bc = _bass.AP(tensor=lnw_d.ap().tensor, offset=0,
                                  ap=[[0, 128], [1, H]])
                lnb_bc = _bass.AP(tensor=lnb_d.ap().tensor, offset=0,
                                  ap=[[0, 128], [1, H]])
                lnw_t = cp.tile([128, H], F32)
                nc.sync.dma_start(lnw_t[:], lnw_bc)
                lnb_t = cp.tile([128, H], F32)
                nc.sync.dma_start(lnb_t[:], lnb_bc)

            qsc = float(1.0 / np.sqrt(H // NH))

            for _rep in range(repeat):
                # vn_all[tok, tok-chunk, d]: v in [token, feature] layout,
                # SBUF-resident across phases 1-2.
                vstack = tc.tile_pool(name="vall", bufs=1)
                vap = vstack.__enter__()
                vn_all = vap.tile([128, T // 128, H], BF16, name="vn_all")

                # -------- Phase 1: stats + QKV projection (fused LN) --------
                with tc.tile_pool(name="xts", bufs=1) as xp, \
                     tc.tile_pool(name="bcast", bufs=1) as bcp:
                    xTs = [xp.tile([128, KO, 512], BF16, name=f"xT{i}")
                           for i in range(TC)]
                    for tch in range(TC):
                        for ko in range(KO):
                            nc.sync.dma_start(
                                xTs[tch][:, ko, :],
                                xT_d.ap()[ko, :,
                                          tch * 512:(tch + 1) * 512])

                    if is_pre:
                        # Sx = 1s@x, Sxx = 1s@x^2: every PSUM partition gets
                        # the per-token sums (broadcast for free).
                        Rs, Rqs, NMRs, NMRqs = [], [], [], []
                        with tc.tile_pool(name="sq", bufs=3) as sqp, \
                             tc.tile_pool(name="stps", bufs=1,
                                          space="PSUM") as stp, \
                             tc.tile_pool(name="stt", bufs=4) as sttp:
                            for tch in range(TC):
                                ps_mu = stp.tile([128, 512], F32,
                                                 name=f"psmu{tch}")
                                ps_v = stp.tile([128, 512], F32,
                                                name=f"psv{tch}")
                                for ko in range(KO):
                                    xsq = sqp.tile([128, 512], BF16)
                                    nc.scalar.activation(
                                        xsq[:], xTs[tch][:, ko, :],
                                        ACT.Square)
                                    nc.tensor.matmul(
                                        ps_mu[:], onesb[:],
                                        xTs[tch][:, ko, :],
                                        start=(ko == 0), stop=(ko == KO - 1))
                                    nc.tensor.matmul(
                                        ps_v[:], onesb[:], xsq[:],
                                        start=(ko == 0), stop=(ko == KO - 1))
                                negmu = sttp.tile([128, 512], F32)
                                nc.vector.tensor_scalar_mul(
                                    negmu[:], ps_mu[:], -1.0 / H)
                                musq = sttp.tile([128, 512], F32)
                                nc.vector.tensor_tensor(
                                    musq[:], negmu[:], negmu[:],
                                    mybir.AluOpType.mult)
                                var = sttp.tile([128, 512], F32)
                                nc.vector.tensor_scalar_mul(
                                    var[:], ps_v[:], 1.0 / H)
                                nc.vector.tensor_tensor(
                                    var[:], var[:], musq[:],
                                    mybir.AluOpType.subtract)
                                # rstd = exp(-0.5*ln(var+eps)): both on the
                                # (idle) Scalar engine, no DVE reciprocal.
                                lnv = sttp.tile([128, 512], F32)
                                nc.scalar.activation(lnv[:], var[:], ACT.Ln,
                                                     bias=eps_t[:], scale=1.0)
                                R = bcp.tile([128, 512], F32, name=f"R{tch}")
                                nc.scalar.activation(R[:], lnv[:], ACT.Exp,
                                                     scale=-0.5)
                                Rq = bcp.tile([128, 512], F32,
                                              name=f"Rq{tch}")
                                nc.vector.tensor_scalar_mul(Rq[:], R[:], qsc)
                                NMR = bcp.tile([128, 512], F32,
                                               name=f"NMR{tch}")
                                nc.vector.tensor_tensor(
                                    NMR[:], negmu[:], R[:],
                                    mybir.AluOpType.mult)
                                NMRq = bcp.tile([128, 512], F32,
                                                name=f"NMRq{tch}")
                                nc.vector.tensor_scalar_mul(NMRq[:], NMR[:],
                                                            qsc)
                                Rs.append(R)
                                Rqs.append(Rq)
                                NMRs.append(NMR)
                                NMRqs.append(NMRq)

                    with tc.tile_pool(name="wch", bufs=3) as wp, \
                         tc.tile_pool(name="ev1", bufs=6) as ep, \
                         tc.tile_pool(name="t1p", bufs=3) as t1p, \
                         tc.tile_pool(name="tmp1", bufs=3) as tmp1, \
                         tc.tile_pool(name="ps1", bufs=2, space="PSUM") as pp1, \
                         tc.tile_pool(name="tps1", bufs=2, space="PSUM") as tp1:
                        pending_v = []

                        def flush_v():
                            # PE-transpose a finished v evacuation into
                            # vn_all (emitted one nch later so the DVE
                            # evacuation has time to complete).
                            for ev, nch, tch in pending_v:
                                for c in range(4):
                                    pt = tp1.tile([128, 128], BF16)
                                    nc.tensor.transpose(
                                        pt[:], ev[:, c * 128:(c + 1) * 128],
                                        identb[:])
                                    nc.vector.tensor_copy(
                                        vn_all[:, tch * 4 + c,
                                               (nch - NQK) * 128:
                                               (nch - NQK + 1) * 128], pt[:])
                            pending_v.clear()

                        for nch in range(NCH):
                            wt = wp.tile([128, KO, 128], BF16)
                            nc.scalar.dma_start(
                                wt[:], qkv_ap[:, :, nch * 128:(nch + 1) * 128])
                            for tch in range(TC):
                                ps = pp1.tile([128, 512], F32,
                                              tag=f"ps1_{tch}",
                                              name=f"ps1_{tch}")
                                for ko in range(KO):
                                    nc.tensor.matmul(
                                        ps[:], wt[:, ko], xTs[tch][:, ko, :],
                                        start=(ko == 0), stop=(ko == KO - 1))
                                flush_v()
                                ev = ep.tile([128, 512], BF16)
                                if is_pre:
                                    isq = nch * 128 < H
                                    Rsel = (Rqs if isq else Rs)[tch]
                                    NMRsel = (NMRqs if isq else NMRs)[tch]
                                    t1 = t1p.tile([128, 512], F32)
                                    if has_bias:
                                        nc.vector.tensor_scalar(
                                            out=t1[:], in0=NMRsel[:],
                                            scalar1=cs_t[:, nch:nch + 1],
                                            scalar2=bq_t[:, nch:nch + 1],
                                            op0=mybir.AluOpType.mult,
                                            op1=mybir.AluOpType.add)
                                    else:
                                        nc.vector.tensor_scalar_mul(
                                            t1[:], NMRsel[:],
                                            cs_t[:, nch:nch + 1])
                                    tmp = tmp1.tile([128, 512], F32)
                                    nc.vector.tensor_tensor(
                                        tmp[:], ps[:], Rsel[:],
                                        mybir.AluOpType.mult)
                                    nc.vector.tensor_tensor(
                                        ev[:], tmp[:], t1[:],
                                        mybir.AluOpType.add)
                                elif nch * 128 < H:
                                    nc.vector.tensor_scalar_mul(ev[:], ps[:],
                                                                qsc)
                                else:
                                    nc.vector.tensor_copy(ev[:], ps[:])
                                if nch < NQK:
                                    nc.sync.dma_start(
                                        projT[nch * 128:(nch + 1) * 128,
                                              tch * 512:(tch + 1) * 512],
                                        ev[:])
                                else:
                                    pending_v.append((ev, nch, tch))
                        flush_v()

                # ---------------- Phase 2: attention ----------------
                with tc.tile_pool(name="ctxt", bufs=1) as cxp:
                    # Half-token tiles: phase 3 on tokens 0-511 (batches 0-1)
                    # starts while attention runs batches 2-3.
                    ctxTs = [cxp.tile([128, KO, 512], BF16, name=f"ctxT{i}")
                             for i in range(TC)]
                    assert NH % 2 == 0 and B_core % 2 == 0
                    # phase-3 weight pool opened alongside attention: all
                    # o-weight chunks prefetch on the scalar queue (which
                    # only carries exp activations during attention).
                    p3stack = tc.tile_pool(name="och", bufs=2)
                    op_ = p3stack.__enter__()
                    ots_pre = []
                    # groups: (bh, p) = batch-half x head-pair; each group
                    # loads q,k for 2 heads x 512 tokens as one DMA each.
                    groups = [(bh, p) for bh in range(B_core // 2)
                              for p in range(NH // 2)]
                    # pairs: two per group (the two batches in the half)
                    pairs = [(g, bs) for g in range(len(groups))
                             for bs in range(2)]
                    with tc.tile_pool(name="ld2", bufs=2) as ld, \
                         tc.tile_pool(name="exp2", bufs=2) as xpp, \
                         tc.tile_pool(name="rec2", bufs=2) as rp, \
                         tc.tile_pool(name="ps2s", bufs=2, space="PSUM") as p2s, \
                         tc.tile_pool(name="ps2m", bufs=2, space="PSUM") as p2m, \
                         tc.tile_pool(name="ps2c", bufs=2, space="PSUM") as p2c, \
                         tc.tile_pool(name="lnps", bufs=2, space="PSUM") as lnp:
                        gtt = {}
                        stt = {}

                        def emit_load_group(g):
                            bh, p = groups[g]
                            q_ = ld.tile([128, 2 * DT, 512], BF16, tag="qT")
                            k_ = ld.tile([128, 2 * DT, 512], BF16, tag="kT")
                            r0 = 2 * p * D
                            nc.sync.dma_start(
                                q_[:],
                                projT[r0:r0 + 2 * D,
                                      bh * 512:(bh + 1) * 512]
                                .rearrange("(c p) t -> p c t", p=128))
                            nc.sync.dma_start(
                                k_[:],
                                projT[H + r0:H + r0 + 2 * D,
                                      bh * 512:(bh + 1) * 512]
                                .rearrange("(c p) t -> p c t", p=128))
                            gtt[g] = dict(q=q_, k=k_)

                        def emit_produce(i):
                            g, bs = pairs[i]
                            bh, p = groups[g]
                            b = 2 * bh + bs
                            gt = gtt[g]
                            expT = xpp.tile([128, KT, 2 * S], BF16, tag="expT")
                            for kt in range(KT):
                                pss = p2s.tile([128, 2 * S], F32)
                                for h in range(2):
                                    for dt in range(DT):
                                        c = h * DT + dt
                                        nc.tensor.matmul(
                                            pss[:, h * S:(h + 1) * S],
                                            gt["k"][:, c,
                                                    bs * S + kt * 128:
                                                    bs * S + (kt + 1) * 128],
                                            gt["q"][:, c,
                                                    bs * S:(bs + 1) * S],
                                            start=(dt == 0),
                                            stop=(dt == DT - 1))
                                nc.scalar.activation(
                                    expT[:, kt], pss[:], ACT.Exp,
                                    bias=mb_t[:, b * KT + kt:b * KT + kt + 1],
                                    scale=1.0)
                            stt[i] = dict(b=b, p=p, expT=expT)

                        def emit_sumexp(i):
                            st = stt[i]
                            psm = p2m.tile([128, 2 * S], F32)
                            for kt in range(KT):
                                nc.tensor.matmul(psm[:], onesb[:],
                                                 st["expT"][:, kt],
                                                 start=(kt == 0),
                                                 stop=(kt == KT - 1))
                            # 1/sumexp = exp(-ln(sumexp)) on the Scalar
                            # engine: keeps the (bottleneck) Vector engine
                            # free for the ctx evacuations.
                            lnm = lnp.tile([128, 2 * S], F32)
                            nc.scalar.activation(lnm[:], psm[:], ACT.Ln)
                            rec = rp.tile([128, 2 * S], F32, tag="rec")
                            nc.scalar.activation(rec[:], lnm[:], ACT.Exp,
                                                 scale=-1.0)
                            st["rec"] = rec

                        def emit_consume(i):
                            st = stt.pop(i)
                            b, p = st["b"], st["p"]
                            expT, rec = st["expT"], st["rec"]
                            for dt in range(DT):
                                psc = p2c.tile([128, 2 * S], F32)
                                for h in range(2):
                                    n = 2 * p + h
                                    for kt in range(KT):
                                        nc.tensor.matmul(
                                            psc[:, h * S:(h + 1) * S],
                                            vn_all[:, b * KT + kt,
                                                   n * D + dt * 128:
                                                   n * D + (dt + 1) * 128],
                                            expT[:, kt, h * S:(h + 1) * S],
                                            start=(kt == 0), stop=(kt == KT - 1))
                                for h in range(2):
                                    n = 2 * p + h
                                    nc.vector.tensor_tensor(
                                        ctxTs[b // 2][:, n * DT + dt,
                                                      (b % 2) * S:
                                                      (b % 2 + 1) * S],
                                        psc[:, h * S:(h + 1) * S],
                                        rec[:, h * S:(h + 1) * S],
                                        mybir.AluOpType.mult)

                        NPAIR = len(pairs)
                        emit_load_group(0)
                        emit_load_group(1)
                        emit_produce(0)
                        for i in range(NPAIR):
                            # sumexp first: its reciprocal runs on DVE while
                            # the PE streams the next pair's scores, so the
                            # ctx matmuls in emit_consume never wait on it.
                            emit_sumexp(i)
                            g, bs = pairs[i]
                            if bs == 0 and g + 2 < len(groups):
                                emit_load_group(g + 2)
                            if i == 0:
                                # prefetch o-weight chunks on the scalar
                                # queue (idle but for exps in phase 2).
                                for hoch in range(2):
                                    ot = op_.tile([128, KO, 128], BF16,
                                                  tag="ot")
                                    nc.gpsimd.dma_start(
                                        ot[:],
                                        o_ap[:, :,
                                             hoch * 128:(hoch + 1) * 128])
                                    ots_pre.append(ot)
                            if i + 1 < NPAIR:
                                emit_produce(i + 1)
                            emit_consume(i)

                    # ---------------- Phase 3: output projection ----------------
                    with tc.tile_pool(name="ev3", bufs=3) as e3, \
                         tc.tile_pool(name="ps3", bufs=2, space="PSUM") as pp3:
                        for hoch in range(KO):
                            if hoch < 2:
                                ot = ots_pre[hoch]
                            else:
                                ot = op_.tile([128, KO, 128], BF16, tag="ot")
                                nc.gpsimd.dma_start(
                                    ot[:],
                                    o_ap[:, :, hoch * 128:(hoch + 1) * 128])
                            psl = [pp3.tile([128, 512], F32, tag=f"ps3_{t}",
                                            name=f"ps3_{t}")
                                   for t in range(TC)]
                            for tch in range(TC):
                                for ko in range(KO):
                                    nc.tensor.matmul(
                                        psl[tch][:], ot[:, ko],
                                        ctxTs[tch][:, ko, :],
                                        start=(ko == 0), stop=(ko == KO - 1))
                            for tch in range(TC):
                                ps = psl[tch]
                                ev = e3.tile([128, 512], F32)
                                nc.vector.tensor_copy(ev[:], ps[:])
                                dst = (out_d.ap() if is_pre else oTs)
                                nc.sync.dma_start(
                                    dst[hoch * 128:(hoch + 1) * 128,
                                        tch * 512:(tch + 1) * 512], ev[:])
                    p3stack.__exit__(None, None, None)
                vstack.__exit__(None, None, None)

                # ---------------- Phase 4 (isPre=0): transpose + post-LN -------
                if not is_pre:
                    with tc.tile_pool(name="p4in", bufs=3) as p4i, \
                         tc.tile_pool(name="p4out", bufs=2) as p4o, \
                         tc.tile_pool(name="st4", bufs=8) as st4, \
                         tc.tile_pool(name="sq4", bufs=2) as sq4, \
                         tc.tile_pool(name="tps4", bufs=4, space="PSUM") as tp4:
                        for tt in range(T // 128):
                            on = p4o.tile([128, H], F32)
                            for hh in range(KO):
                                it = p4i.tile([128, 128], F32)
                                nc.sync.dma_start(
                                    it[:], oTs[hh * 128:(hh + 1) * 128,
                                               tt * 128:(tt + 1) * 128])
                                pt = tp4.tile([128, 128], F32)
                                nc.tensor.transpose(pt[:], it[:], ident[:])
                                nc.vector.tensor_copy(
                                    on[:, hh * 128:(hh + 1) * 128], pt[:])
                            ssum = st4.tile([128, 1], F32)
                            nc.vector.reduce_sum(out=ssum[:], in_=on[:],
                                                 axis=mybir.AxisListType.X)
                            negmu = st4.tile([128, 1], F32)
                            nc.vector.tensor_scalar_mul(negmu[:], ssum[:], -1.0 / H)
                            xsq = sq4.tile([128, H], F32)
                            vsum = st4.tile([128, 1], F32)
                            nc.scalar.activation(xsq[:], on[:], ACT.Square,
                                                 bias=negmu[:], scale=1.0,
                                                 accum_out=vsum[:])
                            sd = st4.tile([128, 1], F32)
                            nc.scalar.activation(sd[:], vsum[:], ACT.Sqrt,
                                                 bias=eps_t[:], scale=1.0 / H)
                            rstd = st4.tile([128, 1], F32)
                            nc.vector.reciprocal(rstd[:], sd[:])
                            nc.vector.tensor_scalar(
                                out=on[:], in0=on[:],
                                scalar1=negmu[:], scalar2=rstd[:],
                                op0=mybir.AluOpType.add,
                                op1=mybir.AluOpType.mult)
                            nc.vector.tensor_tensor(on[:], on[:], lnw_t[:],
                                                    mybir.AluOpType.mult)
                            nc.vector.tensor_tensor(on[:], on[:], lnb_t[:],
                                                    mybir.AluOpType.add)
                            nc.sync.dma_start(
                                out_d.ap()[tt * 128:(tt + 1) * 128, :], on[:])

    nc.finalize()
    return nc


@lru_cache(maxsize=4)
def _get_runner(n_cores, T, S, H, NH, is_pre, has_bias, repeat=1):
    """Build + jit once; returns fn(in_maps) -> list of out dicts."""
    import jax
    import numpy as _np
    from jax.sharding import Mesh, PartitionSpec
    from jax.experimental.shard_map import shard_map
    import concourse.mybir as mybir
    from concourse import bass2jax
    from concourse.bass2jax import _bass_exec_p, install_neuronx_cc_hook

    nc = _build(n_cores, T, S, H, NH, is_pre, has_bias, repeat)
    install_neuronx_cc_hook()

    partition_name = (nc.partition_id_tensor.name
                      if nc.partition_id_tensor else None)
    in_names, out_names, out_avals, zero_shapes = [], [], [], []
    for alloc in nc.m.functions[0].allocations:
        if not isinstance(alloc, mybir.MemoryLocationSet):
            continue
        name = alloc.memorylocations[0].name
        if alloc.kind == "ExternalInput":
            if name != partition_name:
                in_names.append(name)
        elif alloc.kind == "ExternalOutput":
            out_names.append(name)
            shape = tuple(alloc.tensor_shape)
            dtype = mybir.dt.np(alloc.dtype)
            out_avals.append(jax.core.ShapedArray(shape, dtype))
            zero_shapes.append((shape, dtype))
    n_params = len(in_names)
    n_outs = len(out_avals)
    all_in_names = list(in_names) + list(out_names)
    if partition_name is not None:
        all_in_names.append(partition_name)

    def _body(*args):
        operands = list(args)
        if partition_name is not None:
            operands.append(bass2jax.partition_id_tensor())
        outs = _bass_exec_p.bind(
            *operands,
            out_avals=tuple(out_avals),
            in_names=tuple(all_in_names),
            out_names=tuple(out_names),
            lowering_input_output_aliases=(),
            sim_require_finite=True,
            sim_require_nnan=True,
            nc=nc,
        )
        return tuple(outs)

    devices = jax.devices()[:n_cores]
    if n_cores == 1:
        jfn = jax.jit(_body, keep_unused=True)

        def _prep(in_maps):
            args = [jax.device_put(_np.asarray(in_maps[0][n]))
                    for n in in_names]
            zeros = [jax.device_put(_np.zeros(s, d)) for s, d in zero_shapes]
            return args + zeros

        def _collect(outs):
            return [{n: _np.asarray(outs[i]) for i, n in enumerate(out_names)}]
    else:
        mesh = Mesh(np.asarray(devices), ("core",))
        from jax.sharding import NamedSharding
        shard = NamedSharding(mesh, PartitionSpec("core"))
        repl = NamedSharding(mesh, PartitionSpec())
        REPLICATED = {"qkvw", "ow", "bqkv", "lnw", "lnb", "csum"}
        in_specs = tuple(
            (PartitionSpec() if n in REPLICATED else PartitionSpec("core"))
            for n in in_names) + (PartitionSpec("core"),) * n_outs
        out_specs = (PartitionSpec("core"),) * n_outs
        jfn = jax.jit(
            shard_map(_body, mesh=mesh, in_specs=in_specs,
                      out_specs=out_specs, check_rep=False),
            keep_unused=True)

        def _prep(in_maps):
            concat_in = []
            for n in in_names:
                if n in REPLICATED:
                    concat_in.append(
                        jax.device_put(_np.asarray(in_maps[0][n]), repl))
                else:
                    concat_in.append(jax.device_put(
                        _np.concatenate([_np.asarray(m[n]) for m in in_maps],
                                        axis=0), shard))
            zeros = [
                jax.device_put(
                    _np.zeros((n_cores * s[0], *s[1:]), d), shard)
                for s, d in zero_shapes]
            return concat_in + zeros

        def _collect(outs):
            return [
                {n: _np.asarray(outs[i]).reshape(
                    n_cores, *out_avals[i].shape)[c]
                 for i, n in enumerate(out_names)}
                for c in range(n_cores)]

    class Runner:
        in_names_ = in_names
        out_names_ = out_names

        def prep(self, in_maps):
            return _prep(in_maps)

        def call(self, args):
            return jfn(*args)

        def run(self, in_maps):
            outs = jfn(*_prep(in_maps))
            jax.block_until_ready(outs)
            return _collect(outs)

        def collect(self, outs):
            return _collect(outs)

    return Runner()


def _prep_core_inputs(inp, mask, weight, bias, qkv, o, is_pre, n_cores,
                      NH=16):
    """Host-side prep: fold LN weight into qkv, pre-transpose x per core,
    build per-core input dicts."""
    import ml_dtypes
    B, S, H = inp.shape
    D = H // NH
    B_core = B // n_cores
    T = B_core * S
    KO = H // 128
    H3 = 3 * H
    KT = S // 128

    # Pre-LN: xn = z*w + b with z the normalized input, so
    # xn @ qkv = z @ (w[:,None]*qkv) + (b @ qkv): fold w into the weights
    # and b into a per-output-channel additive term applied on-device.
    # The mean subtraction uses (x-mu)@W = x@W - mu*colsum(W), with
    # colsum computed here from the bf16-rounded weights (exact algebra).
    qkvw = qkv.astype(np.float32)
    if is_pre:
        w = weight.astype(np.float32)
        if not np.all(w == 1.0):
            qkvw = qkvw * w[:, None]
        bqkv = bias.astype(np.float32) @ qkv.astype(np.float32)
    else:
        bqkv = np.zeros(H3, dtype=np.float32)
    bqkv[:H] *= np.float32(1.0 / np.sqrt(D))
    has_bias = bool(np.any(bqkv))

    qkv_b = qkvw.astype(ml_dtypes.bfloat16)
    csum = qkv_b.astype(np.float64).sum(axis=0).astype(np.float32)  # [H3]
    qkv_r = qkv_b.reshape(KO, 128, H3)
    o_r = o.astype(ml_dtypes.bfloat16).reshape(KO, 128, H)

    maskbias = np.where(mask != 0, np.float32(NEG_BIG), np.float32(0.0))
    maskbias = maskbias.astype(np.float32)  # [B, S]

    in_maps = []
    for c in range(n_cores):
        xb = inp[c * B_core:(c + 1) * B_core].reshape(T, H)
        xbT = np.ascontiguousarray(xb.astype(np.float32).T)  # [H, T]
        mb = maskbias[c * B_core:(c + 1) * B_core].reshape(B_core * KT, 128)
        m = {
            "xT": xbT.reshape(KO, 128, T).astype(ml_dtypes.bfloat16),
            "qkvw": qkv_r,
            "ow": o_r,
            "maskb": np.ascontiguousarray(mb),
        }
        if is_pre:
            m["csum"] = np.ascontiguousarray(csum.reshape(H3 // 128, 128))
        if has_bias:
            m["bqkv"] = np.ascontiguousarray(
                bqkv.reshape(H3 // 128, 128))
        if not is_pre:
            m["lnw"] = np.ascontiguousarray(weight.astype(np.float32))
            m["lnb"] = np.ascontiguousarray(bias.astype(np.float32))
        in_maps.append(m)
    return in_maps, has_bias, (B, S, H, NH, B_core, T)


def kernel(inp, mask, weight, bias, qkv, o, isPre):
    inp = np.asarray(inp)
    mask = np.asarray(mask)
    weight = np.asarray(weight)
    bias = np.asarray(bias)
    qkv = np.asarray(qkv)
    o = np.asarray(o)
    is_pre = bool(int(np.asarray(isPre)))

    n_cores = 8
    NH = 16
    in_maps, has_bias, (B, S, H, _, B_core, T) = _prep_core_inputs(
        inp, mask, weight, bias, qkv, o, is_pre, n_cores)

    runner = _get_runner(n_cores, T, S, H, NH, is_pre, has_bias)
    results = runner.run(in_maps)

    out = np.empty((B, S, H), dtype=np.float32)
    for c in range(n_cores):
        if is_pre:
            outT = results[c]["outT"]  # [H, T]
            out[c * B_core:(c + 1) * B_core] = outT.T.reshape(B_core, S, H)
        else:
            out[c * B_core:(c + 1) * B_core] = (
                results[c]["outN"].reshape(B_core, S, H))
    return out


# revision 22
# speedup vs baseline: 1.0387x; 1.0387x over previous
"""Trainium2 Bass kernel for nn_MultiHeadLayer (pre-LN MHA, fused QKV).

Self-contained: takes FULL inputs, shards data-parallel over batch across
8 NeuronCores, runs a Bass/Tile kernel per core, gathers the full output.

Per-core dataflow (T = B_core*S tokens, H hidden, NH heads, D = H/NH):
  Phase 1: host supplies xT [H, T] bf16 (pre-transposed). LN without PE
           transposes: Sx = 1s @ xT and Sxx = 1s @ xT^2 ones-matmuls land
           the per-token mean/var broadcast across partitions in PSUM for
           free. The mean is NOT subtracted from x; instead the identity
           (x - mu) @ W = x @ W - mu * colsum(W) is applied at the PSUM
           evacuation of the QKV matmul together with the 1/std factor
           (colsum(W) precomputed on host). All-bf16 weight-stationary
           matmuls. q,k stream to projT [2H, T] bf16 in DRAM; v is
           PE-transposed at evacuation time into an SBUF-resident
           vn_all [tok, d] so phase 2 needs no v traffic at all.
  Phase 2: per (batch, head): scoresT = kT.T @ qT (k on partitions), exp
           fused with additive mask via per-partition ACT bias, sumexp via
           ones-matmul, ctxT from vn_all with the normalization fused into
           the PSUM evacuation. q,k are loaded per head-pair x batch-half
           group as single large row-contiguous DMAs on the sync queue;
           the scalar queue carries only exp + o-weight prefetch.
  Phase 3: outT = o.T @ ctxT (bf16), weight-stationary -> outT [H, T] ->
           host transposes during unshard.
"""

import numpy as np
from functools import lru_cache

LN_EPS = 1e-5
NEG_BIG = -1.0e30


def _build(n_cores, T, S, H, NH, is_pre, has_bias, repeat=1, kt_js=None):
    import concourse.bacc as bacc
    import concourse.mybir as mybir
    import concourse.tile as tile
    from concourse.masks import make_identity

    F32 = mybir.dt.float32
    BF16 = mybir.dt.bfloat16
    ACT = mybir.ActivationFunctionType

    KO = H // 128          # hidden-dim 128-chunks
    H3 = 3 * H
    D = H // NH
    DT = D // 128          # d-chunks per head
    KT = S // 128          # key-token 128-chunks per sequence
    B_core = T // S
    TC = T // 512          # token 512-chunks
    NCH = H3 // 128        # qkv column chunks of 128
    NQK = 2 * H // 128     # q+k column chunks (written to DRAM)

    # Key compaction: kt_js[j] = number of 128-key chunks kept for batch
    # slot j (host packs each slot's unmasked keys first, zero-padded).
    # k/v are computed over the compacted TG columns only, then scattered
    # back to the dense per-slot layout so phase 2 is compaction-agnostic.
    if kt_js is None:
        kt_js = (KT,) * B_core
    kt_js = tuple(kt_js)
    assert len(kt_js) == B_core and all(1 <= k <= KT for k in kt_js)
    NKC = sum(kt_js)
    TG = NKC * 128
    blk2dense = []
    for j in range(B_core):
        for kt in range(kt_js[j]):
            blk2dense.append(j * KT + kt)
    missing = [ch for ch in range(B_core * KT) if ch not in set(blk2dense)]
    # PSUM-sized column chunks over the compacted key tokens
    tgw, tgo, r = [], [], 0
    while r < TG:
        w = min(512, TG - r)
        tgw.append(w)
        tgo.append(r)
        r += w
    # contiguous dense-scatter runs per chunk: (src_block, dense_chunk, n)
    kruns = []
    for i, w in enumerate(tgw):
        b0 = tgo[i] // 128
        runs, s = [], 0
        while s < w // 128:
            d0 = blk2dense[b0 + s]
            n = 1
            while (s + n < w // 128
                   and blk2dense[b0 + s + n] == d0 + n):
                n += 1
            runs.append((s, d0, n))
            s += n
        kruns.append(runs)

    nc = bacc.Bacc("TRN2", target_bir_lowering=False, debug=False,
                   num_devices=n_cores)

    xT_d = nc.dram_tensor("xT", [KO, 128, T], BF16, kind="ExternalInput")
    xTg_d = nc.dram_tensor("xTg", [KO, 128, TG], BF16, kind="ExternalInput")
    qkv_d = nc.dram_tensor("qkvw", [KO, 128, H3], BF16, kind="ExternalInput")
    o_d = nc.dram_tensor("ow", [KO, 128, H], BF16, kind="ExternalInput")
    # maskb[b*KT+kt, :] = additive key-mask bias for key tokens kt*128..+128
    mb_d = nc.dram_tensor("maskb", [B_core * KT, 128], F32,
                          kind="ExternalInput")
    if is_pre:
        # csum[i, :] = sum_h qkvw_bf16[h, i*128:(i+1)*128] (host, exact)
        cs_d = nc.dram_tensor("csum", [NCH, 128], F32, kind="ExternalInput")
    if has_bias:
        # bqkv[i, :] = (bias @ qkvw)[i*128:(i+1)*128]
        bq_d = nc.dram_tensor("bqkv", [NCH, 128], F32, kind="ExternalInput")
    if is_pre:
        out_d = nc.dram_tensor("outT", [H, T], F32, kind="ExternalOutput")
    else:
        # post-LN needs LN params applied on-device to the output rows
        lnw_d = nc.dram_tensor("lnw", [H], F32, kind="ExternalInput")
        lnb_d = nc.dram_tensor("lnb", [H], F32, kind="ExternalInput")
        out_d = nc.dram_tensor("outN", [T, H], F32, kind="ExternalOutput")

    with tile.TileContext(nc) as tc:
        with tc.tile_pool(name="consts", bufs=1) as cp, \
             tc.tile_pool(name="dram", bufs=1, space="DRAM") as dp:
            ident = cp.tile([128, 128], F32)
            make_identity(nc, ident[:])
            identb = cp.tile([128, 128], BF16)
            nc.vector.tensor_copy(identb[:], ident[:])
            onesb = cp.tile([128, 128], BF16)
            nc.vector.memset(onesb[:], 1.0)
            eps_t = cp.tile([128, 1], F32)
            nc.vector.memset(eps_t[:], LN_EPS)
            mb_t = cp.tile([128, B_core * KT], F32)
            nc.sync.dma_start(mb_t[:], mb_d.ap().rearrange("i p -> p i"))
            if is_pre:
                cs_t = cp.tile([128, NCH], F32)
                nc.sync.dma_start(cs_t[:], cs_d.ap().rearrange("i p -> p i"))
            if has_bias:
                bq_t = cp.tile([128, NCH], F32)
                nc.sync.dma_start(bq_t[:], bq_d.ap().rearrange("i p -> p i"))

            qkv_ap = qkv_d.ap().rearrange("ko p n -> p ko n")
            o_ap = o_d.ap().rearrange("ko p n -> p ko n")
            projT = dp.tile([2 * H, T], BF16)
            if not is_pre:
                oTs = dp.tile([H, T], F32)
                import concourse.bass as _bass
                lnw_bc = _bass.AP(tensor=lnw_d.ap().tensor, offset=0,
                                  ap=[[0, 128], [1, H]])
                lnb_bc = _bass.AP(tensor=lnb_d.ap().tensor, offset=0,
                                  ap=[[0, 128], [1, H]])
                lnw_t = cp.tile([128, H], F32)
                nc.sync.dma_start(lnw_t[:], lnw_bc)
                lnb_t = cp.tile([128, H], F32)
                nc.sync.dma_start(lnb_t[:], lnb_bc)

            qsc = float(1.0 / np.sqrt(H // NH))

            def emit_stats(xt, w, sqp, stp, sttp, bcp, tagn, want_q):
                """ones-matmul LN stats over one rhs chunk [128, KO, w];
                returns (Rsel, NMRsel) with the q scale folded if want_q."""
                ps_mu = stp.tile([128, w], F32, tag="psmu")
                ps_v = stp.tile([128, w], F32, tag="psv")
                for ko in range(KO):
                    xsq = sqp.tile([128, w], BF16)
                    nc.scalar.activation(xsq[:], xt[:, ko, :], ACT.Square)
                    nc.tensor.matmul(ps_mu[:], onesb[:], xt[:, ko, :],
                                     start=(ko == 0), stop=(ko == KO - 1))
                    nc.tensor.matmul(ps_v[:], onesb[:], xsq[:],
                                     start=(ko == 0), stop=(ko == KO - 1))
                negmu = sttp.tile([128, w], F32)
                nc.vector.tensor_scalar_mul(negmu[:], ps_mu[:], -1.0 / H)
                musq = sttp.tile([128, w], F32)
                nc.vector.tensor_tensor(musq[:], negmu[:], negmu[:],
                                        mybir.AluOpType.mult)
                var = sttp.tile([128, w], F32)
                nc.vector.tensor_scalar_mul(var[:], ps_v[:], 1.0 / H)
                nc.vector.tensor_tensor(var[:], var[:], musq[:],
                                        mybir.AluOpType.subtract)
                # rstd = exp(-0.5*ln(var+eps)): both on the (idle) Scalar
                # engine, no DVE reciprocal.
                lnv = sttp.tile([128, w], F32)
                nc.scalar.activation(lnv[:], var[:], ACT.Ln,
                                     bias=eps_t[:], scale=1.0)
                R = bcp.tile([128, w], F32, name=f"R{tagn}")
                nc.scalar.activation(R[:], lnv[:], ACT.Exp, scale=-0.5)
                if want_q:
                    Rq = bcp.tile([128, w], F32, name=f"Rq{tagn}")
                    nc.vector.tensor_scalar_mul(Rq[:], R[:], qsc)
                    NMRq = bcp.tile([128, w], F32, name=f"NMRq{tagn}")
                    nc.vector.tensor_tensor(NMRq[:], negmu[:], Rq[:],
                                            mybir.AluOpType.mult)
                    return Rq, NMRq
                NMR = bcp.tile([128, w], F32, name=f"NMR{tagn}")
                nc.vector.tensor_tensor(NMR[:], negmu[:], R[:],
                                        mybir.AluOpType.mult)
                return R, NMR

            def emit_evac(ev, ps, Rsel, NMRsel, nch, is_q):
                if is_pre:
                    t1 = t1p.tile(list(ev.shape), F32)
                    if has_bias:
                        nc.vector.tensor_scalar(
                            out=t1[:], in0=NMRsel[:],
                            scalar1=cs_t[:, nch:nch + 1],
                            scalar2=bq_t[:, nch:nch + 1],
                            op0=mybir.AluOpType.mult,
                            op1=mybir.AluOpType.add)
                    else:
                        nc.vector.tensor_scalar_mul(
                            t1[:], NMRsel[:], cs_t[:, nch:nch + 1])
                    tmp = tmp1.tile(list(ev.shape), F32)
                    nc.vector.tensor_tensor(tmp[:], ps[:], Rsel[:],
                                            mybir.AluOpType.mult)
                    nc.vector.tensor_tensor(ev[:], tmp[:], t1[:],
                                            mybir.AluOpType.add)
                elif is_q:
                    nc.vector.tensor_scalar_mul(ev[:], ps[:], qsc)
                else:
                    nc.vector.tensor_copy(ev[:], ps[:])

            for _rep in range(repeat):
                # vn_all[tok, dense tok-chunk, d]: v in [token, feature]
                # layout, SBUF-resident across phases 1-2.
                vstack = tc.tile_pool(name="vall", bufs=1)
                vap = vstack.__enter__()
                vn_all = vap.tile([128, T // 128, H], BF16, name="vn_all")

                # ---- Phase 1a: q projection, per token-half (fused LN) ----
                gstack = tc.tile_pool(name="xtg", bufs=1)
                gp_ = gstack.__enter__()
                xTgs = [gp_.tile([128, KO, w], BF16, name=f"xTg{i}")
                        for i, w in enumerate(tgw)]
                for tch in range(TC):
                    with tc.tile_pool(name="xts", bufs=1) as xp, \
                         tc.tile_pool(name="bcq", bufs=1) as bcp:
                        xt = xp.tile([128, KO, 512], BF16, name=f"xT{tch}")
                        for ko in range(KO):
                            nc.sync.dma_start(
                                xt[:, ko, :],
                                xT_d.ap()[ko, :, tch * 512:(tch + 1) * 512])
                        if tch == 0:
                            # compacted kv tokens: transfer ordered behind
                            # the first q half, well before the kv pass.
                            for i, w in enumerate(tgw):
                                for ko in range(KO):
                                    nc.sync.dma_start(
                                        xTgs[i][:, ko, :],
                                        xTg_d.ap()[ko, :,
                                                   tgo[i]:tgo[i] + w])
                        Rsel = NMRsel = None
                        if is_pre:
                            with tc.tile_pool(name="sq", bufs=3) as sqp, \
                                 tc.tile_pool(name="stps", bufs=2,
                                              space="PSUM") as stp, \
                                 tc.tile_pool(name="stt", bufs=4) as sttp:
                                Rsel, NMRsel = emit_stats(
                                    xt, 512, sqp, stp, sttp, bcp,
                                    f"q{tch}", True)
                        with tc.tile_pool(name="wch", bufs=2) as wp, \
                             tc.tile_pool(name="ev1", bufs=4) as ep, \
                             tc.tile_pool(name="t1p", bufs=2) as t1p, \
                             tc.tile_pool(name="tmp1", bufs=2) as tmp1, \
                             tc.tile_pool(name="ps1", bufs=2,
                                          space="PSUM") as pp1:
                            for nch in range(KO):
                                wt = wp.tile([128, KO, 128], BF16)
                                nc.scalar.dma_start(
                                    wt[:],
                                    qkv_ap[:, :, nch * 128:(nch + 1) * 128])
                                ps = pp1.tile([128, 512], F32)
                                for ko in range(KO):
                                    nc.tensor.matmul(
                                        ps[:], wt[:, ko], xt[:, ko, :],
                                        start=(ko == 0), stop=(ko == KO - 1))
                                ev = ep.tile([128, 512], BF16)
                                emit_evac(ev, ps, Rsel, NMRsel, nch, True)
                                nc.sync.dma_start(
                                    projT[nch * 128:(nch + 1) * 128,
                                          tch * 512:(tch + 1) * 512], ev[:])

                # ---- Phase 1b: k,v over compacted keys, dense scatter ----
                with tc.tile_pool(name="bcg", bufs=1) as bcg, \
                     tc.tile_pool(name="zp", bufs=1) as zp:
                    Rg, NMRg = [], []
                    if is_pre:
                        with tc.tile_pool(name="sqg", bufs=3) as sqp, \
                             tc.tile_pool(name="stpsg", bufs=2,
                                          space="PSUM") as stp, \
                             tc.tile_pool(name="sttg", bufs=4) as sttp:
                            for i, w in enumerate(tgw):
                                r_, n_ = emit_stats(xTgs[i], w, sqp, stp,
                                                    sttp, bcg, f"g{i}", False)
                                Rg.append(r_)
                                NMRg.append(n_)
                    # zero-fill the dense k/v chunks with no compacted
                    # source: their keys are fully masked (exp -> 0), the
                    # zeros only keep the scores finite.
                    if missing:
                        zev = zp.tile([128, 128], BF16)
                        nc.vector.memset(zev[:], 0.0)
                        for nch in range(KO, NQK):
                            for ch in missing:
                                nc.sync.dma_start(
                                    projT[nch * 128:(nch + 1) * 128,
                                          ch * 128:(ch + 1) * 128], zev[:])
                        for ch in missing:
                            nc.vector.memset(vn_all[:, ch, :], 0.0)

                    with tc.tile_pool(name="wch2", bufs=2) as wp, \
                         tc.tile_pool(name="ev1g", bufs=6) as ep, \
                         tc.tile_pool(name="t1p", bufs=3) as t1p, \
                         tc.tile_pool(name="tmp1", bufs=3) as tmp1, \
                         tc.tile_pool(name="ps1g", bufs=2,
                                      space="PSUM") as pp1, \
                         tc.tile_pool(name="tps1", bufs=2,
                                      space="PSUM") as tp1:
                        pending_v = []

                        def flush_v():
                            # PE-transpose a finished v evacuation into its
                            # dense vn_all chunks (emitted one nch later so
                            # the DVE evacuation has time to complete).
                            for ev, nch, i in pending_v:
                                for s in range(tgw[i] // 128):
                                    dch = blk2dense[tgo[i] // 128 + s]
                                    pt = tp1.tile([128, 128], BF16)
                                    nc.tensor.transpose(
                                        pt[:], ev[:, s * 128:(s + 1) * 128],
                                        identb[:])
                                    nc.vector.tensor_copy(
                                        vn_all[:, dch,
                                               (nch - NQK) * 128:
                                               (nch - NQK + 1) * 128], pt[:])
                            pending_v.clear()

                        for nch in range(KO, NCH):
                            wt = wp.tile([128, KO, 128], BF16)
                            nc.scalar.dma_start(
                                wt[:], qkv_ap[:, :, nch * 128:(nch + 1) * 128])
                            for i, w in enumerate(tgw):
                                ps = pp1.tile([128, w], F32,
                                              tag=f"ps1_{i}",
                                              name=f"ps1_{i}")
                                for ko in range(KO):
                                    nc.tensor.matmul(
                                        ps[:], wt[:, ko], xTgs[i][:, ko, :],
                                        start=(ko == 0), stop=(ko == KO - 1))
                                flush_v()
                                ev = ep.tile([128, w], BF16, tag=f"ev{i}")
                                emit_evac(ev, ps,
                                          Rg[i] if is_pre else None,
                                          NMRg[i] if is_pre else None,
                                          nch, False)
                                if nch < NQK:
                                    # scatter compact blocks to their dense
                                    # column positions (contiguous runs)
                                    for s0, d0, nb in kruns[i]:
                                        nc.sync.dma_start(
                                            projT[nch * 128:(nch + 1) * 128,
                                                  d0 * 128:
                                                  (d0 + nb) * 128],
                                            ev[:, s0 * 128:(s0 + nb) * 128])
                                else:
                                    pending_v.append((ev, nch, i))
                        flush_v()
                gstack.__exit__(None, None, None)

                # ---------------- Phase 2: attention ----------------
                with tc.tile_pool(name="ctxt", bufs=1) as cxp:
                    # Half-token tiles: phase 3 on tokens 0-511 (batches 0-1)
                    # starts while attention runs batches 2-3.
                    ctxTs = [cxp.tile([128, KO, 512], BF16, name=f"ctxT{i}")
                             for i in range(TC)]
                    assert NH % 2 == 0 and B_core % 2 == 0
                    # phase-3 weight pool opened alongside attention: all
                    # o-weight chunks prefetch on the scalar queue (which
                    # only carries exp activations during attention).
                    p3stack = tc.tile_pool(name="och", bufs=2)
                    op_ = p3stack.__enter__()
                    ots_pre = []
                    # groups: (bh, p) = batch-half x head-pair; each group
                    # loads q,k for 2 heads x 512 tokens as one DMA each.
                    groups = [(bh, p) for bh in range(B_core // 2)
                              for p in range(NH // 2)]
                    # pairs: two per group (the two batches in the half)
                    pairs = [(g, bs) for g in range(len(groups))
                             for bs in range(2)]
                    with tc.tile_pool(name="ld2", bufs=2) as ld, \
                         tc.tile_pool(name="exp2", bufs=2) as xpp, \
                         tc.tile_pool(name="rec2", bufs=2) as rp, \
                         tc.tile_pool(name="ps2s", bufs=2, space="PSUM") as p2s, \
                         tc.tile_pool(name="ps2m", bufs=2, space="PSUM") as p2m, \
                         tc.tile_pool(name="ps2c", bufs=2, space="PSUM") as p2c, \
                         tc.tile_pool(name="lnps", bufs=2, space="PSUM") as lnp:
                        gtt = {}
                        stt = {}

                        def emit_load_group(g):
                            bh, p = groups[g]
                            q_ = ld.tile([128, 2 * DT, 512], BF16, tag="qT")
                            k_ = ld.tile([128, 2 * DT, 512], BF16, tag="kT")
                            r0 = 2 * p * D
                            nc.sync.dma_start(
                                q_[:],
                                projT[r0:r0 + 2 * D,
                                      bh * 512:(bh + 1) * 512]
                                .rearrange("(c p) t -> p c t", p=128))
                            nc.sync.dma_start(
                                k_[:],
                                projT[H + r0:H + r0 + 2 * D,
                                      bh * 512:(bh + 1) * 512]
                                .rearrange("(c p) t -> p c t", p=128))
                            gtt[g] = dict(q=q_, k=k_)

                        def emit_produce(i):
                            g, bs = pairs[i]
                            bh, p = groups[g]
                            b = 2 * bh + bs
                            gt = gtt[g]
                            expT = xpp.tile([128, KT, 2 * S], BF16, tag="expT")
                            for kt in range(KT):
                                pss = p2s.tile([128, 2 * S], F32)
                                for h in range(2):
                                    for dt in range(DT):
                                        c = h * DT + dt
                                        nc.tensor.matmul(
                                            pss[:, h * S:(h + 1) * S],
                                            gt["k"][:, c,
                                                    bs * S + kt * 128:
                                                    bs * S + (kt + 1) * 128],
                                            gt["q"][:, c,
                                                    bs * S:(bs + 1) * S],
                                            start=(dt == 0),
                                            stop=(dt == DT - 1))
                                nc.scalar.activation(
                                    expT[:, kt], pss[:], ACT.Exp,
                                    bias=mb_t[:, b * KT + kt:b * KT + kt + 1],
                                    scale=1.0)
                            stt[i] = dict(b=b, p=p, expT=expT)

                        def emit_sumexp(i):
                            st = stt[i]
                            psm = p2m.tile([128, 2 * S], F32)
                            for kt in range(KT):
                                nc.tensor.matmul(psm[:], onesb[:],
                                                 st["expT"][:, kt],
                                                 start=(kt == 0),
                                                 stop=(kt == KT - 1))
                            # 1/sumexp = exp(-ln(sumexp)) on the Scalar
                            # engine: keeps the (bottleneck) Vector engine
                            # free for the ctx evacuations.
                            lnm = lnp.tile([128, 2 * S], F32)
                            nc.scalar.activation(lnm[:], psm[:], ACT.Ln)
                            rec = rp.tile([128, 2 * S], F32, tag="rec")
                            nc.scalar.activation(rec[:], lnm[:], ACT.Exp,
                                                 scale=-1.0)
                            st["rec"] = rec

                        def emit_consume(i):
                            st = stt.pop(i)
                            b, p = st["b"], st["p"]
                            expT, rec = st["expT"], st["rec"]
                            for dt in range(DT):
                                psc = p2c.tile([128, 2 * S], F32)
                                for h in range(2):
                                    n = 2 * p + h
                                    for kt in range(KT):
                                        nc.tensor.matmul(
                                            psc[:, h * S:(h + 1) * S],
                                            vn_all[:, b * KT + kt,
                                                   n * D + dt * 128:
                                                   n * D + (dt + 1) * 128],
                                            expT[:, kt, h * S:(h + 1) * S],
                                            start=(kt == 0), stop=(kt == KT - 1))
                                for h in range(2):
                                    n = 2 * p + h
                                    nc.vector.tensor_tensor(
                                        ctxTs[b // 2][:, n * DT + dt,
                                                      (b % 2) * S:
                                                      (b % 2 + 1) * S],
                                        psc[:, h * S:(h + 1) * S],
                                        rec[:, h * S:(h + 1) * S],
                                        mybir.AluOpType.mult)

                        NPAIR = len(pairs)
                        emit_load_group(0)
                        emit_load_group(1)
                        emit_produce(0)
                        for i in range(NPAIR):
                            # sumexp first: its reciprocal runs on DVE while
                            # the PE streams the next pair's scores, so the
                            # ctx matmuls in emit_consume never wait on it.
                            emit_sumexp(i)
                            g, bs = pairs[i]
                            if bs == 0 and g + 2 < len(groups):
                                emit_load_group(g + 2)
                            if i == 0:
                                # prefetch o-weight chunks on the scalar
                                # queue (idle but for exps in phase 2).
                                for hoch in range(2):
                                    ot = op_.tile([128, KO, 128], BF16,
                                                  tag="ot")
                                    nc.scalar.dma_start(
                                        ot[:],
                                        o_ap[:, :,
                                             hoch * 128:(hoch + 1) * 128])
                                    ots_pre.append(ot)
                            if i + 1 < NPAIR:
                                emit_produce(i + 1)
                            emit_consume(i)

                    # ---------------- Phase 3: output projection ----------------
                    with tc.tile_pool(name="ev3", bufs=3) as e3, \
                         tc.tile_pool(name="ps3", bufs=2, space="PSUM") as pp3:
                        for hoch in range(KO):
                            if hoch < 2:
                                ot = ots_pre[hoch]
                            else:
                                ot = op_.tile([128, KO, 128], BF16, tag="ot")
                                nc.scalar.dma_start(
                                    ot[:],
                                    o_ap[:, :, hoch * 128:(hoch + 1) * 128])
                            psl = [pp3.tile([128, 512], F32, tag=f"ps3_{t}",
                                            name=f"ps3_{t}")
                                   for t in range(TC)]
                            for tch in range(TC):
                                for ko in range(KO):
                                    nc.tensor.matmul(
                                        psl[tch][:], ot[:, ko],
                                        ctxTs[tch][:, ko, :],
                                        start=(ko == 0), stop=(ko == KO - 1))
                            for tch in range(TC):
                                ps = psl[tch]
                                ev = e3.tile([128, 512], F32)
                                nc.vector.tensor_copy(ev[:], ps[:])
                                dst = (out_d.ap() if is_pre else oTs)
                                nc.sync.dma_start(
                                    dst[hoch * 128:(hoch + 1) * 128,
                                        tch * 512:(tch + 1) * 512], ev[:])
                    p3stack.__exit__(None, None, None)
                vstack.__exit__(None, None, None)

                # ---------------- Phase 4 (isPre=0): transpose + post-LN -------
                if not is_pre:
                    with tc.tile_pool(name="p4in", bufs=3) as p4i, \
                         tc.tile_pool(name="p4out", bufs=2) as p4o, \
                         tc.tile_pool(name="st4", bufs=8) as st4, \
                         tc.tile_pool(name="sq4", bufs=2) as sq4, \
                         tc.tile_pool(name="tps4", bufs=4, space="PSUM") as tp4:
                        for tt in range(T // 128):
                            on = p4o.tile([128, H], F32)
                            for hh in range(KO):
                                it = p4i.tile([128, 128], F32)
                                nc.sync.dma_start(
                                    it[:], oTs[hh * 128:(hh + 1) * 128,
                                               tt * 128:(tt + 1) * 128])
                                pt = tp4.tile([128, 128], F32)
                                nc.tensor.transpose(pt[:], it[:], ident[:])
                                nc.vector.tensor_copy(
                                    on[:, hh * 128:(hh + 1) * 128], pt[:])
                            ssum = st4.tile([128, 1], F32)
                            nc.vector.reduce_sum(out=ssum[:], in_=on[:],
                                                 axis=mybir.AxisListType.X)
                            negmu = st4.tile([128, 1], F32)
                            nc.vector.tensor_scalar_mul(negmu[:], ssum[:], -1.0 / H)
                            xsq = sq4.tile([128, H], F32)
                            vsum = st4.tile([128, 1], F32)
                            nc.scalar.activation(xsq[:], on[:], ACT.Square,
                                                 bias=negmu[:], scale=1.0,
                                                 accum_out=vsum[:])
                            sd = st4.tile([128, 1], F32)
                            nc.scalar.activation(sd[:], vsum[:], ACT.Sqrt,
                                                 bias=eps_t[:], scale=1.0 / H)
                            rstd = st4.tile([128, 1], F32)
                            nc.vector.reciprocal(rstd[:], sd[:])
                            nc.vector.tensor_scalar(
                                out=on[:], in0=on[:],
                                scalar1=negmu[:], scalar2=rstd[:],
                                op0=mybir.AluOpType.add,
                                op1=mybir.AluOpType.mult)
                            nc.vector.tensor_tensor(on[:], on[:], lnw_t[:],
                                                    mybir.AluOpType.mult)
                            nc.vector.tensor_tensor(on[:], on[:], lnb_t[:],
                                                    mybir.AluOpType.add)
                            nc.sync.dma_start(
                                out_d.ap()[tt * 128:(tt + 1) * 128, :], on[:])

    nc.finalize()
    return nc


@lru_cache(maxsize=4)
def _get_runner(n_cores, T, S, H, NH, is_pre, has_bias, repeat=1,
                kt_js=None):
    """Build + jit once; returns fn(in_maps) -> list of out dicts."""
    import jax
    import numpy as _np
    from jax.sharding import Mesh, PartitionSpec
    from jax.experimental.shard_map import shard_map
    import concourse.mybir as mybir
    from concourse import bass2jax
    from concourse.bass2jax import _bass_exec_p, install_neuronx_cc_hook

    nc = _build(n_cores, T, S, H, NH, is_pre, has_bias, repeat, kt_js=kt_js)
    install_neuronx_cc_hook()

    partition_name = (nc.partition_id_tensor.name
                      if nc.partition_id_tensor else None)
    in_names, out_names, out_avals, zero_shapes = [], [], [], []
    for alloc in nc.m.functions[0].allocations:
        if not isinstance(alloc, mybir.MemoryLocationSet):
            continue
        name = alloc.memorylocations[0].name
        if alloc.kind == "ExternalInput":
            if name != partition_name:
                in_names.append(name)
        elif alloc.kind == "ExternalOutput":
            out_names.append(name)
            shape = tuple(alloc.tensor_shape)
            dtype = mybir.dt.np(alloc.dtype)
            out_avals.append(jax.core.ShapedArray(shape, dtype))
            zero_shapes.append((shape, dtype))
    n_params = len(in_names)
    n_outs = len(out_avals)
    all_in_names = list(in_names) + list(out_names)
    if partition_name is not None:
        all_in_names.append(partition_name)

    def _body(*args):
        operands = list(args)
        if partition_name is not None:
            operands.append(bass2jax.partition_id_tensor())
        outs = _bass_exec_p.bind(
            *operands,
            out_avals=tuple(out_avals),
            in_names=tuple(all_in_names),
            out_names=tuple(out_names),
            lowering_input_output_aliases=(),
            sim_require_finite=True,
            sim_require_nnan=True,
            nc=nc,
        )
        return tuple(outs)

    devices = jax.devices()[:n_cores]
    if n_cores == 1:
        jfn = jax.jit(_body, keep_unused=True)

        def _prep(in_maps):
            args = [jax.device_put(_np.asarray(in_maps[0][n]))
                    for n in in_names]
            zeros = [jax.device_put(_np.zeros(s, d)) for s, d in zero_shapes]
            return args + zeros

        def _collect(outs):
            return [{n: _np.asarray(outs[i]) for i, n in enumerate(out_names)}]
    else:
        mesh = Mesh(np.asarray(devices), ("core",))
        from jax.sharding import NamedSharding
        shard = NamedSharding(mesh, PartitionSpec("core"))
        repl = NamedSharding(mesh, PartitionSpec())
        REPLICATED = {"qkvw", "ow", "bqkv", "lnw", "lnb", "csum"}
        in_specs = tuple(
            (PartitionSpec() if n in REPLICATED else PartitionSpec("core"))
            for n in in_names) + (PartitionSpec("core"),) * n_outs
        out_specs = (PartitionSpec("core"),) * n_outs
        jfn = jax.jit(
            shard_map(_body, mesh=mesh, in_specs=in_specs,
                      out_specs=out_specs, check_rep=False),
            keep_unused=True)

        def _prep(in_maps):
            concat_in = []
            for n in in_names:
                if n in REPLICATED:
                    concat_in.append(
                        jax.device_put(_np.asarray(in_maps[0][n]), repl))
                else:
                    concat_in.append(jax.device_put(
                        _np.concatenate([_np.asarray(m[n]) for m in in_maps],
                                        axis=0), shard))
            zeros = [
                jax.device_put(
                    _np.zeros((n_cores * s[0], *s[1:]), d), shard)
                for s, d in zero_shapes]
            return concat_in + zeros

        def _collect(outs):
            return [
                {n: _np.asarray(outs[i]).reshape(
                    n_cores, *out_avals[i].shape)[c]
                 for i, n in enumerate(out_names)}
                for c in range(n_cores)]

    class Runner:
        in_names_ = in_names
        out_names_ = out_names

        def prep(self, in_maps):
            return _prep(in_maps)

        def call(self, args):
            return jfn(*args)

        def run(self, in_maps):
            outs = jfn(*_prep(in_maps))
            jax.block_until_ready(outs)
            return _collect(outs)

        def collect(self, outs):
            return _collect(outs)

    return Runner()


def _prep_core_inputs(inp, mask, weight, bias, qkv, o, is_pre, n_cores,
                      NH=16):
    """Host-side prep: fold LN weight into qkv, pre-transpose x per core,
    build per-core input dicts."""
    import ml_dtypes
    B, S, H = inp.shape
    D = H // NH
    B_core = B // n_cores
    T = B_core * S
    KO = H // 128
    H3 = 3 * H
    KT = S // 128

    # Pre-LN: xn = z*w + b with z the normalized input, so
    # xn @ qkv = z @ (w[:,None]*qkv) + (b @ qkv): fold w into the weights
    # and b into a per-output-channel additive term applied on-device.
    # The mean subtraction uses (x-mu)@W = x@W - mu*colsum(W), with
    # colsum computed here from the bf16-rounded weights (exact algebra).
    qkvw = qkv.astype(np.float32)
    if is_pre:
        w = weight.astype(np.float32)
        if not np.all(w == 1.0):
            qkvw = qkvw * w[:, None]
        bqkv = bias.astype(np.float32) @ qkv.astype(np.float32)
    else:
        bqkv = np.zeros(H3, dtype=np.float32)
    bqkv[:H] *= np.float32(1.0 / np.sqrt(D))
    has_bias = bool(np.any(bqkv))

    qkv_b = qkvw.astype(ml_dtypes.bfloat16)
    csum = qkv_b.astype(np.float64).sum(axis=0).astype(np.float32)  # [H3]
    qkv_r = qkv_b.reshape(KO, 128, H3)
    o_r = o.astype(ml_dtypes.bfloat16).reshape(KO, 128, H)

    # Key compaction: rank batches by unmasked-key count (descending) and
    # deal them round-robin so slot j across all cores needs only
    # kt_js[j] = ceil(max_U(slot j)/128) key chunks (SPMD: one program).
    mask = np.asarray(mask)
    U = (mask == 0).sum(axis=1).astype(np.int64)  # unmasked keys per batch
    order = np.argsort(-U, kind="stable")
    KTF = S // 128
    kt_js = []
    for j in range(B_core):
        mx = int(U[order[j * n_cores:(j + 1) * n_cores]].max())
        kt_js.append(int(min(KTF, max(1, -(-mx // 128)))))
    kt_js = tuple(kt_js)
    TG = 128 * sum(kt_js)

    in_maps, bidx_all = [], []
    for c in range(n_cores):
        bidx = [int(order[j * n_cores + c]) for j in range(B_core)]
        bidx_all.append(bidx)
        xb = inp[bidx].reshape(T, H)
        xbT = np.ascontiguousarray(xb.astype(np.float32).T)  # [H, T]
        # compacted kv tokens: per slot, unmasked keys first, zero-padded
        gs = []
        for j, b in enumerate(bidx):
            idx = np.nonzero(mask[b] == 0)[0]
            g = inp[b][idx].astype(np.float32)
            pad = kt_js[j] * 128 - g.shape[0]
            if pad > 0:
                g = np.concatenate(
                    [g, np.zeros((pad, H), np.float32)], axis=0)
            gs.append(g)
        xg = np.concatenate(gs, axis=0)  # [TG, H]
        xgT = np.ascontiguousarray(xg.T)
        # additive mask over the dense per-slot key layout: real compacted
        # keys -> 0, padding and unused tail chunks -> NEG_BIG
        mb = np.zeros((B_core * KTF, 128), np.float32)
        for j, b in enumerate(bidx):
            for kt in range(KTF):
                nreal = (min(128, max(0, int(U[b]) - kt * 128))
                         if kt < kt_js[j] else 0)
                mb[j * KTF + kt, nreal:] = NEG_BIG
        m = {
            "xT": xbT.reshape(KO, 128, T).astype(ml_dtypes.bfloat16),
            "xTg": xgT.reshape(KO, 128, TG).astype(ml_dtypes.bfloat16),
            "qkvw": qkv_r,
            "ow": o_r,
            "maskb": np.ascontiguousarray(mb),
        }
        if is_pre:
            m["csum"] = np.ascontiguousarray(csum.reshape(H3 // 128, 128))
        if has_bias:
            m["bqkv"] = np.ascontiguousarray(
                bqkv.reshape(H3 // 128, 128))
        if not is_pre:
            m["lnw"] = np.ascontiguousarray(weight.astype(np.float32))
            m["lnb"] = np.ascontiguousarray(bias.astype(np.float32))
        in_maps.append(m)
    return in_maps, has_bias, (B, S, H, NH, B_core, T), {
        "kt_js": kt_js, "bidx_all": bidx_all}


def kernel(inp, mask, weight, bias, qkv, o, isPre):
    inp = np.asarray(inp)
    mask = np.asarray(mask)
    weight = np.asarray(weight)
    bias = np.asarray(bias)
    qkv = np.asarray(qkv)
    o = np.asarray(o)
    is_pre = bool(int(np.asarray(isPre)))

    n_cores = 8
    NH = 16
    in_maps, has_bias, (B, S, H, _, B_core, T), extra = _prep_core_inputs(
        inp, mask, weight, bias, qkv, o, is_pre, n_cores)

    runner = _get_runner(n_cores, T, S, H, NH, is_pre, has_bias,
                         kt_js=extra["kt_js"])
    results = runner.run(in_maps)

    out = np.empty((B, S, H), dtype=np.float32)
    for c in range(n_cores):
        if is_pre:
            outT = results[c]["outT"]  # [H, T]
            slab = outT.T.reshape(B_core, S, H)
        else:
            slab = results[c]["outN"].reshape(B_core, S, H)
        for j, b in enumerate(extra["bidx_all"][c]):
            out[b] = slab[j]
    return out


# revision 28
# speedup vs baseline: 1.0543x; 1.0150x over previous
"""Trainium2 Bass kernel for nn_MultiHeadLayer (pre-LN MHA, fused QKV).

Self-contained: takes FULL inputs, shards data-parallel over batch across
8 NeuronCores, runs a Bass/Tile kernel per core, gathers the full output.

Per-core dataflow (T = B_core*S tokens, H hidden, NH heads, D = H/NH):
  Phase 1: host supplies xT [H, T] bf16 (pre-transposed). LN without PE
           transposes: Sx = 1s @ xT and Sxx = 1s @ xT^2 ones-matmuls land
           the per-token mean/var broadcast across partitions in PSUM for
           free. The mean is NOT subtracted from x; instead the identity
           (x - mu) @ W = x @ W - mu * colsum(W) is applied at the PSUM
           evacuation of the QKV matmul together with the 1/std factor
           (colsum(W) precomputed on host). All-bf16 weight-stationary
           matmuls. q,k stream to projT [2H, T] bf16 in DRAM; v is
           PE-transposed at evacuation time into an SBUF-resident
           vn_all [tok, d] so phase 2 needs no v traffic at all.
  Phase 2: per (batch, head): scoresT = kT.T @ qT (k on partitions), exp
           fused with additive mask via per-partition ACT bias, sumexp via
           ones-matmul, ctxT from vn_all with the normalization fused into
           the PSUM evacuation. q,k are loaded per head-pair x batch-half
           group as single large row-contiguous DMAs on the sync queue;
           the scalar queue carries only exp + o-weight prefetch.
  Phase 3: outT = o.T @ ctxT (bf16), weight-stationary -> outT [H, T] ->
           host transposes during unshard.
"""

import numpy as np
from functools import lru_cache

LN_EPS = 1e-5
NEG_BIG = -1.0e30


def _build(n_cores, T, S, H, NH, is_pre, has_bias, repeat=1, kt_js=None):
    import concourse.bacc as bacc
    import concourse.mybir as mybir
    import concourse.tile as tile
    from concourse.masks import make_identity

    F32 = mybir.dt.float32
    BF16 = mybir.dt.bfloat16
    ACT = mybir.ActivationFunctionType

    KO = H // 128          # hidden-dim 128-chunks
    H3 = 3 * H
    D = H // NH
    DT = D // 128          # d-chunks per head
    KT = S // 128          # key-token 128-chunks per sequence
    B_core = T // S
    TC = T // 512          # token 512-chunks
    NCH = H3 // 128        # qkv column chunks of 128
    NQK = 2 * H // 128     # q+k column chunks (written to DRAM)

    # Key compaction: kt_js[j] = number of 128-key chunks kept for batch
    # slot j (host packs each slot's unmasked keys first, zero-padded).
    # k/v are computed over the compacted TG columns only, then scattered
    # back to the dense per-slot layout so phase 2 is compaction-agnostic.
    if kt_js is None:
        kt_js = (KT,) * B_core
    kt_js = tuple(kt_js)
    assert len(kt_js) == B_core and all(1 <= k <= KT for k in kt_js)
    NKC = sum(kt_js)
    TG = NKC * 128
    blk2dense = []
    for j in range(B_core):
        for kt in range(kt_js[j]):
            blk2dense.append(j * KT + kt)
    missing = [ch for ch in range(B_core * KT) if ch not in set(blk2dense)]
    # PSUM-sized column chunks over the compacted key tokens
    tgw, tgo, r = [], [], 0
    while r < TG:
        w = min(512, TG - r)
        tgw.append(w)
        tgo.append(r)
        r += w
    # contiguous dense-scatter runs per chunk: (src_block, dense_chunk, n)
    kruns = []
    for i, w in enumerate(tgw):
        b0 = tgo[i] // 128
        runs, s = [], 0
        while s < w // 128:
            d0 = blk2dense[b0 + s]
            n = 1
            while (s + n < w // 128
                   and blk2dense[b0 + s + n] == d0 + n):
                n += 1
            runs.append((s, d0, n))
            s += n
        kruns.append(runs)

    nc = bacc.Bacc("TRN2", target_bir_lowering=False, debug=False,
                   num_devices=n_cores)

    xT_d = nc.dram_tensor("xT", [KO, 128, T], BF16, kind="ExternalInput")
    xTg_d = nc.dram_tensor("xTg", [KO, 128, TG], BF16, kind="ExternalInput")
    qkv_d = nc.dram_tensor("qkvw", [KO, 128, H3], BF16, kind="ExternalInput")
    o_d = nc.dram_tensor("ow", [KO, 128, H], BF16, kind="ExternalInput")
    # maskb[b*KT+kt, :] = additive key-mask bias for key tokens kt*128..+128
    mb_d = nc.dram_tensor("maskb", [B_core * KT, 128], F32,
                          kind="ExternalInput")
    if is_pre:
        # csum[i, :] = sum_h qkvw_bf16[h, i*128:(i+1)*128] (host, exact)
        cs_d = nc.dram_tensor("csum", [NCH, 128], F32, kind="ExternalInput")
    if has_bias:
        # bqkv[i, :] = (bias @ qkvw)[i*128:(i+1)*128]
        bq_d = nc.dram_tensor("bqkv", [NCH, 128], F32, kind="ExternalInput")
    if is_pre:
        out_d = nc.dram_tensor("outT", [H, T], F32, kind="ExternalOutput")
    else:
        # post-LN needs LN params applied on-device to the output rows
        lnw_d = nc.dram_tensor("lnw", [H], F32, kind="ExternalInput")
        lnb_d = nc.dram_tensor("lnb", [H], F32, kind="ExternalInput")
        out_d = nc.dram_tensor("outN", [T, H], F32, kind="ExternalOutput")

    with tile.TileContext(nc) as tc:
        with tc.tile_pool(name="consts", bufs=1) as cp, \
             tc.tile_pool(name="dram", bufs=1, space="DRAM") as dp:
            ident = cp.tile([128, 128], F32)
            make_identity(nc, ident[:])
            identb = cp.tile([128, 128], BF16)
            nc.vector.tensor_copy(identb[:], ident[:])
            onesb = cp.tile([128, 128], BF16)
            nc.vector.memset(onesb[:], 1.0)
            eps_t = cp.tile([128, 1], F32)
            nc.vector.memset(eps_t[:], LN_EPS)
            mb_t = cp.tile([128, B_core * KT], F32)
            nc.sync.dma_start(mb_t[:], mb_d.ap().rearrange("i p -> p i"))
            if is_pre:
                cs_t = cp.tile([128, NCH], F32)
                nc.sync.dma_start(cs_t[:], cs_d.ap().rearrange("i p -> p i"))
            if has_bias:
                bq_t = cp.tile([128, NCH], F32)
                nc.sync.dma_start(bq_t[:], bq_d.ap().rearrange("i p -> p i"))

            qkv_ap = qkv_d.ap().rearrange("ko p n -> p ko n")
            o_ap = o_d.ap().rearrange("ko p n -> p ko n")
            projT = dp.tile([2 * H, T], BF16)
            if not is_pre:
                oTs = dp.tile([H, T], F32)
                import concourse.bass as _bass
                lnw_bc = _bass.AP(tensor=lnw_d.ap().tensor, offset=0,
                                  ap=[[0, 128], [1, H]])
                lnb_bc = _bass.AP(tensor=lnb_d.ap().tensor, offset=0,
                                  ap=[[0, 128], [1, H]])
                lnw_t = cp.tile([128, H], F32)
                nc.sync.dma_start(lnw_t[:], lnw_bc)
                lnb_t = cp.tile([128, H], F32)
                nc.sync.dma_start(lnb_t[:], lnb_bc)

            qsc = float(1.0 / np.sqrt(H // NH))

            def emit_stats(xt, w, sqp, stp, sttp, bcp, tagn, want_q):
                """ones-matmul LN stats over one rhs chunk [128, KO, w];
                returns (Rsel, NMRsel) with the q scale folded if want_q."""
                ps_mu = stp.tile([128, w], F32, tag="psmu")
                ps_v = stp.tile([128, w], F32, tag="psv")
                for ko in range(KO):
                    # square on the (idle) Vector engine: keeps the Scalar
                    # queue free to issue weight-chunk DMAs during stats.
                    xsq = sqp.tile([128, w], BF16)
                    nc.vector.tensor_tensor(xsq[:], xt[:, ko, :],
                                            xt[:, ko, :],
                                            mybir.AluOpType.mult)
                    nc.tensor.matmul(ps_mu[:], onesb[:], xt[:, ko, :],
                                     start=(ko == 0), stop=(ko == KO - 1))
                    nc.tensor.matmul(ps_v[:], onesb[:], xsq[:],
                                     start=(ko == 0), stop=(ko == KO - 1))
                negmu = sttp.tile([128, w], F32)
                nc.vector.tensor_scalar_mul(negmu[:], ps_mu[:], -1.0 / H)
                musq = sttp.tile([128, w], F32)
                nc.vector.tensor_tensor(musq[:], negmu[:], negmu[:],
                                        mybir.AluOpType.mult)
                var = sttp.tile([128, w], F32)
                nc.vector.tensor_scalar_mul(var[:], ps_v[:], 1.0 / H)
                nc.vector.tensor_tensor(var[:], var[:], musq[:],
                                        mybir.AluOpType.subtract)
                # rstd = exp(-0.5*ln(var+eps)): both on the (idle) Scalar
                # engine, no DVE reciprocal.
                lnv = sttp.tile([128, w], F32)
                nc.scalar.activation(lnv[:], var[:], ACT.Ln,
                                     bias=eps_t[:], scale=1.0)
                R = bcp.tile([128, w], F32, name=f"R{tagn}")
                nc.scalar.activation(R[:], lnv[:], ACT.Exp, scale=-0.5)
                if want_q:
                    Rq = bcp.tile([128, w], F32, name=f"Rq{tagn}")
                    nc.vector.tensor_scalar_mul(Rq[:], R[:], qsc)
                    NMRq = bcp.tile([128, w], F32, name=f"NMRq{tagn}")
                    nc.vector.tensor_tensor(NMRq[:], negmu[:], Rq[:],
                                            mybir.AluOpType.mult)
                    return Rq, NMRq
                NMR = bcp.tile([128, w], F32, name=f"NMR{tagn}")
                nc.vector.tensor_tensor(NMR[:], negmu[:], R[:],
                                        mybir.AluOpType.mult)
                return R, NMR

            def emit_evac(ev, ps, Rsel, NMRsel, nch, is_q):
                if is_pre:
                    t1 = t1p.tile(list(ev.shape), F32)
                    if has_bias:
                        nc.vector.tensor_scalar(
                            out=t1[:], in0=NMRsel[:],
                            scalar1=cs_t[:, nch:nch + 1],
                            scalar2=bq_t[:, nch:nch + 1],
                            op0=mybir.AluOpType.mult,
                            op1=mybir.AluOpType.add)
                    else:
                        nc.vector.tensor_scalar_mul(
                            t1[:], NMRsel[:], cs_t[:, nch:nch + 1])
                    tmp = tmp1.tile(list(ev.shape), F32)
                    nc.vector.tensor_tensor(tmp[:], ps[:], Rsel[:],
                                            mybir.AluOpType.mult)
                    nc.vector.tensor_tensor(ev[:], tmp[:], t1[:],
                                            mybir.AluOpType.add)
                elif is_q:
                    nc.vector.tensor_scalar_mul(ev[:], ps[:], qsc)
                else:
                    nc.vector.tensor_copy(ev[:], ps[:])

            for _rep in range(repeat):
                # vn_all[tok, dense tok-chunk, d]: v in [token, feature]
                # layout, SBUF-resident across phases 1-2.
                vstack = tc.tile_pool(name="vall", bufs=1)
                vap = vstack.__enter__()
                vn_all = vap.tile([128, T // 128, H], BF16, name="vn_all")

                # ---- Phase 1a: q projection, per token-half (fused LN) ----
                gstack = tc.tile_pool(name="xtg", bufs=1)
                gp_ = gstack.__enter__()
                xTgs = [gp_.tile([128, KO, w], BF16, name=f"xTg{i}")
                        for i, w in enumerate(tgw)]
                bgstack = tc.tile_pool(name="bcg", bufs=1)
                bcg = bgstack.__enter__()
                Rg, NMRg = [], []
                for tch in range(TC):
                    with tc.tile_pool(name="xts", bufs=1) as xp, \
                         tc.tile_pool(name="bcq", bufs=1) as bcp:
                        xt = xp.tile([128, KO, 512], BF16, name=f"xT{tch}")
                        for ko in range(KO):
                            nc.sync.dma_start(
                                xt[:, ko, :],
                                xT_d.ap()[ko, :, tch * 512:(tch + 1) * 512])
                        if tch == 0:
                            # compacted kv tokens: transfer ordered behind
                            # the first q half, well before the kv pass.
                            for i, w in enumerate(tgw):
                                for ko in range(KO):
                                    nc.sync.dma_start(
                                        xTgs[i][:, ko, :],
                                        xTg_d.ap()[ko, :,
                                                   tgo[i]:tgo[i] + w])
                        Rsel = NMRsel = None
                        if is_pre:
                            with tc.tile_pool(name="sq", bufs=3) as sqp, \
                                 tc.tile_pool(name="stps", bufs=2,
                                              space="PSUM") as stp, \
                                 tc.tile_pool(name="stt", bufs=4) as sttp:
                                Rsel, NMRsel = emit_stats(
                                    xt, 512, sqp, stp, sttp, bcp,
                                    f"q{tch}", True)
                        with tc.tile_pool(name="wch", bufs=2) as wp, \
                             tc.tile_pool(name="ev1", bufs=4) as ep, \
                             tc.tile_pool(name="t1p", bufs=2) as t1p, \
                             tc.tile_pool(name="tmp1", bufs=2) as tmp1, \
                             tc.tile_pool(name="ps1", bufs=2,
                                          space="PSUM") as pp1:
                            for nch in range(KO):
                                wt = wp.tile([128, KO, 128], BF16)
                                nc.scalar.dma_start(
                                    wt[:],
                                    qkv_ap[:, :, nch * 128:(nch + 1) * 128])
                                ps = pp1.tile([128, 512], F32)
                                for ko in range(KO):
                                    nc.tensor.matmul(
                                        ps[:], wt[:, ko], xt[:, ko, :],
                                        start=(ko == 0), stop=(ko == KO - 1))
                                ev = ep.tile([128, 512], BF16)
                                emit_evac(ev, ps, Rsel, NMRsel, nch, True)
                                nc.sync.dma_start(
                                    projT[nch * 128:(nch + 1) * 128,
                                          tch * 512:(tch + 1) * 512], ev[:])
                    if tch == 0 and is_pre:
                        # kv stats emitted between the q halves: this PE
                        # work covers the xT-t1 DMA (blocked on the xts
                        # buffer until the t0 projection finishes reading).
                        with tc.tile_pool(name="sqg", bufs=3) as sqp, \
                             tc.tile_pool(name="stpsg", bufs=2,
                                          space="PSUM") as stp, \
                             tc.tile_pool(name="sttg", bufs=4) as sttp:
                            for i, w in enumerate(tgw):
                                r_, n_ = emit_stats(xTgs[i], w, sqp, stp,
                                                    sttp, bcg, f"g{i}", False)
                                Rg.append(r_)
                                NMRg.append(n_)

                # ---- Phase 1b: k,v over compacted keys, dense scatter ----
                with tc.tile_pool(name="zp", bufs=1) as zp:
                    # zero-fill the dense k/v chunks with no compacted
                    # source: their keys are fully masked (exp -> 0), the
                    # zeros only keep the scores finite.
                    if missing:
                        zev = zp.tile([128, 128], BF16)
                        nc.vector.memset(zev[:], 0.0)
                        for nch in range(KO, NQK):
                            for ch in missing:
                                nc.sync.dma_start(
                                    projT[nch * 128:(nch + 1) * 128,
                                          ch * 128:(ch + 1) * 128], zev[:])
                        for ch in missing:
                            nc.vector.memset(vn_all[:, ch, :], 0.0)

                    with tc.tile_pool(name="wch2", bufs=2) as wp, \
                         tc.tile_pool(name="ev1g", bufs=6) as ep, \
                         tc.tile_pool(name="t1p", bufs=3) as t1p, \
                         tc.tile_pool(name="tmp1", bufs=3) as tmp1, \
                         tc.tile_pool(name="ps1g", bufs=2,
                                      space="PSUM") as pp1, \
                         tc.tile_pool(name="tps1", bufs=2,
                                      space="PSUM") as tp1:
                        pending_v = []

                        def flush_v():
                            # PE-transpose a finished v evacuation into its
                            # dense vn_all chunks (emitted one nch later so
                            # the DVE evacuation has time to complete).
                            for ev, nch, i in pending_v:
                                for s in range(tgw[i] // 128):
                                    dch = blk2dense[tgo[i] // 128 + s]
                                    pt = tp1.tile([128, 128], BF16)
                                    nc.tensor.transpose(
                                        pt[:], ev[:, s * 128:(s + 1) * 128],
                                        identb[:])
                                    nc.vector.tensor_copy(
                                        vn_all[:, dch,
                                               (nch - NQK) * 128:
                                               (nch - NQK + 1) * 128], pt[:])
                            pending_v.clear()

                        for nch in range(KO, NCH):
                            wt = wp.tile([128, KO, 128], BF16)
                            nc.scalar.dma_start(
                                wt[:], qkv_ap[:, :, nch * 128:(nch + 1) * 128])
                            for i, w in enumerate(tgw):
                                ps = pp1.tile([128, w], F32,
                                              tag=f"ps1_{i}",
                                              name=f"ps1_{i}")
                                for ko in range(KO):
                                    nc.tensor.matmul(
                                        ps[:], wt[:, ko], xTgs[i][:, ko, :],
                                        start=(ko == 0), stop=(ko == KO - 1))
                                flush_v()
                                ev = ep.tile([128, w], BF16, tag=f"ev{i}")
                                emit_evac(ev, ps,
                                          Rg[i] if is_pre else None,
                                          NMRg[i] if is_pre else None,
                                          nch, False)
                                if nch < NQK:
                                    # scatter compact blocks to their dense
                                    # column positions (contiguous runs)
                                    for s0, d0, nb in kruns[i]:
                                        nc.sync.dma_start(
                                            projT[nch * 128:(nch + 1) * 128,
                                                  d0 * 128:
                                                  (d0 + nb) * 128],
                                            ev[:, s0 * 128:(s0 + nb) * 128])
                                else:
                                    pending_v.append((ev, nch, i))
                        flush_v()
                bgstack.__exit__(None, None, None)
                gstack.__exit__(None, None, None)

                # ---------------- Phase 2: attention ----------------
                with tc.tile_pool(name="ctxt", bufs=1) as cxp:
                    # Half-token tiles: phase 3 on tokens 0-511 (batches 0-1)
                    # starts while attention runs batches 2-3.
                    ctxTs = [cxp.tile([128, KO, 512], BF16, name=f"ctxT{i}")
                             for i in range(TC)]
                    assert NH % 2 == 0 and B_core % 2 == 0
                    # phase-3 weight pool opened alongside attention: all
                    # o-weight chunks prefetch on the scalar queue (which
                    # only carries exp activations during attention).
                    p3stack = tc.tile_pool(name="och", bufs=2)
                    op_ = p3stack.__enter__()
                    ots_pre = []
                    # groups: (bh, p) = batch-half x head-pair; each group
                    # loads q,k for 2 heads x 512 tokens as one DMA each.
                    groups = [(bh, p) for bh in range(B_core // 2)
                              for p in range(NH // 2)]
                    # pairs: two per group (the two batches in the half)
                    pairs = [(g, bs) for g in range(len(groups))
                             for bs in range(2)]
                    with tc.tile_pool(name="ld2", bufs=2) as ld, \
                         tc.tile_pool(name="exp2", bufs=2) as xpp, \
                         tc.tile_pool(name="rec2", bufs=2) as rp, \
                         tc.tile_pool(name="ps2s", bufs=2, space="PSUM") as p2s, \
                         tc.tile_pool(name="ps2m", bufs=2, space="PSUM") as p2m, \
                         tc.tile_pool(name="ps2c", bufs=2, space="PSUM") as p2c, \
                         tc.tile_pool(name="lnps", bufs=2, space="PSUM") as lnp:
                        gtt = {}
                        stt = {}

                        def emit_load_group(g):
                            bh, p = groups[g]
                            q_ = ld.tile([128, 2 * DT, 512], BF16, tag="qT")
                            k_ = ld.tile([128, 2 * DT, 512], BF16, tag="kT")
                            r0 = 2 * p * D
                            # first groups via the idle GPSIMD queue: the
                            # sync queue is still draining phase-1 writes
                            # when attention starts.
                            eng = nc.gpsimd if g < 2 else nc.sync
                            eng.dma_start(
                                q_[:],
                                projT[r0:r0 + 2 * D,
                                      bh * 512:(bh + 1) * 512]
                                .rearrange("(c p) t -> p c t", p=128))
                            eng.dma_start(
                                k_[:],
                                projT[H + r0:H + r0 + 2 * D,
                                      bh * 512:(bh + 1) * 512]
                                .rearrange("(c p) t -> p c t", p=128))
                            gtt[g] = dict(q=q_, k=k_)

                        def emit_produce(i):
                            g, bs = pairs[i]
                            bh, p = groups[g]
                            b = 2 * bh + bs
                            gt = gtt[g]
                            expT = xpp.tile([128, KT, 2 * S], BF16, tag="expT")
                            for kt in range(KT):
                                pss = p2s.tile([128, 2 * S], F32)
                                for h in range(2):
                                    for dt in range(DT):
                                        c = h * DT + dt
                                        nc.tensor.matmul(
                                            pss[:, h * S:(h + 1) * S],
                                            gt["k"][:, c,
                                                    bs * S + kt * 128:
                                                    bs * S + (kt + 1) * 128],
                                            gt["q"][:, c,
                                                    bs * S:(bs + 1) * S],
                                            start=(dt == 0),
                                            stop=(dt == DT - 1))
                                nc.scalar.activation(
                                    expT[:, kt], pss[:], ACT.Exp,
                                    bias=mb_t[:, b * KT + kt:b * KT + kt + 1],
                                    scale=1.0)
                            stt[i] = dict(b=b, p=p, expT=expT)

                        def emit_sumexp(i):
                            st = stt[i]
                            psm = p2m.tile([128, 2 * S], F32)
                            for kt in range(KT):
                                nc.tensor.matmul(psm[:], onesb[:],
                                                 st["expT"][:, kt],
                                                 start=(kt == 0),
                                                 stop=(kt == KT - 1))
                            # 1/sumexp = exp(-ln(sumexp)) on the Scalar
                            # engine: keeps the (bottleneck) Vector engine
                            # free for the ctx evacuations.
                            lnm = lnp.tile([128, 2 * S], F32)
                            nc.scalar.activation(lnm[:], psm[:], ACT.Ln)
                            rec = rp.tile([128, 2 * S], F32, tag="rec")
                            nc.scalar.activation(rec[:], lnm[:], ACT.Exp,
                                                 scale=-1.0)
                            st["rec"] = rec

                        def emit_consume(i):
                            st = stt.pop(i)
                            b, p = st["b"], st["p"]
                            expT, rec = st["expT"], st["rec"]
                            for dt in range(DT):
                                psc = p2c.tile([128, 2 * S], F32)
                                for h in range(2):
                                    n = 2 * p + h
                                    for kt in range(KT):
                                        nc.tensor.matmul(
                                            psc[:, h * S:(h + 1) * S],
                                            vn_all[:, b * KT + kt,
                                                   n * D + dt * 128:
                                                   n * D + (dt + 1) * 128],
                                            expT[:, kt, h * S:(h + 1) * S],
                                            start=(kt == 0), stop=(kt == KT - 1))
                                for h in range(2):
                                    n = 2 * p + h
                                    nc.vector.tensor_tensor(
                                        ctxTs[b // 2][:, n * DT + dt,
                                                      (b % 2) * S:
                                                      (b % 2 + 1) * S],
                                        psc[:, h * S:(h + 1) * S],
                                        rec[:, h * S:(h + 1) * S],
                                        mybir.AluOpType.mult)

                        NPAIR = len(pairs)
                        emit_load_group(0)
                        emit_load_group(1)
                        emit_produce(0)
                        for i in range(NPAIR):
                            # sumexp first: its reciprocal runs on DVE while
                            # the PE streams the next pair's scores, so the
                            # ctx matmuls in emit_consume never wait on it.
                            emit_sumexp(i)
                            g, bs = pairs[i]
                            if bs == 0 and g + 2 < len(groups):
                                emit_load_group(g + 2)
                            if i == 0:
                                # prefetch o-weight chunks on the scalar
                                # queue (idle but for exps in phase 2).
                                for hoch in range(2):
                                    ot = op_.tile([128, KO, 128], BF16,
                                                  tag="ot")
                                    nc.gpsimd.dma_start(
                                        ot[:],
                                        o_ap[:, :,
                                             hoch * 128:(hoch + 1) * 128])
                                    ots_pre.append(ot)
                            if i + 1 < NPAIR:
                                emit_produce(i + 1)
                            emit_consume(i)

                    # ---------------- Phase 3: output projection ----------------
                    with tc.tile_pool(name="ev3", bufs=3) as e3, \
                         tc.tile_pool(name="ps3", bufs=2, space="PSUM") as pp3:
                        for hoch in range(KO):
                            if hoch < 2:
                                ot = ots_pre[hoch]
                            else:
                                ot = op_.tile([128, KO, 128], BF16, tag="ot")
                                nc.gpsimd.dma_start(
                                    ot[:],
                                    o_ap[:, :, hoch * 128:(hoch + 1) * 128])
                            psl = [pp3.tile([128, 512], F32, tag=f"ps3_{t}",
                                            name=f"ps3_{t}")
                                   for t in range(TC)]
                            for tch in range(TC):
                                for ko in range(KO):
                                    nc.tensor.matmul(
                                        psl[tch][:], ot[:, ko],
                                        ctxTs[tch][:, ko, :],
                                        start=(ko == 0), stop=(ko == KO - 1))
                            for tch in range(TC):
                                ps = psl[tch]
                                ev = e3.tile([128, 512], F32)
                                nc.vector.tensor_copy(ev[:], ps[:])
                                dst = (out_d.ap() if is_pre else oTs)
                                nc.sync.dma_start(
                                    dst[hoch * 128:(hoch + 1) * 128,
                                        tch * 512:(tch + 1) * 512], ev[:])
                    p3stack.__exit__(None, None, None)
                vstack.__exit__(None, None, None)

                # ---------------- Phase 4 (isPre=0): transpose + post-LN -------
                if not is_pre:
                    with tc.tile_pool(name="p4in", bufs=3) as p4i, \
                         tc.tile_pool(name="p4out", bufs=2) as p4o, \
                         tc.tile_pool(name="st4", bufs=8) as st4, \
                         tc.tile_pool(name="sq4", bufs=2) as sq4, \
                         tc.tile_pool(name="tps4", bufs=4, space="PSUM") as tp4:
                        for tt in range(T // 128):
                            on = p4o.tile([128, H], F32)
                            for hh in range(KO):
                                it = p4i.tile([128, 128], F32)
                                nc.sync.dma_start(
                                    it[:], oTs[hh * 128:(hh + 1) * 128,
                                               tt * 128:(tt + 1) * 128])
                                pt = tp4.tile([128, 128], F32)
                                nc.tensor.transpose(pt[:], it[:], ident[:])
                                nc.vector.tensor_copy(
                                    on[:, hh * 128:(hh + 1) * 128], pt[:])
                            ssum = st4.tile([128, 1], F32)
                            nc.vector.reduce_sum(out=ssum[:], in_=on[:],
                                                 axis=mybir.AxisListType.X)
                            negmu = st4.tile([128, 1], F32)
                            nc.vector.tensor_scalar_mul(negmu[:], ssum[:], -1.0 / H)
                            xsq = sq4.tile([128, H], F32)
                            vsum = st4.tile([128, 1], F32)
                            nc.scalar.activation(xsq[:], on[:], ACT.Square,
                                                 bias=negmu[:], scale=1.0,
                                                 accum_out=vsum[:])
                            sd = st4.tile([128, 1], F32)
                            nc.scalar.activation(sd[:], vsum[:], ACT.Sqrt,
                                                 bias=eps_t[:], scale=1.0 / H)
                            rstd = st4.tile([128, 1], F32)
                            nc.vector.reciprocal(rstd[:], sd[:])
                            nc.vector.tensor_scalar(
                                out=on[:], in0=on[:],
                                scalar1=negmu[:], scalar2=rstd[:],
                                op0=mybir.AluOpType.add,
                                op1=mybir.AluOpType.mult)
                            nc.vector.tensor_tensor(on[:], on[:], lnw_t[:],
                                                    mybir.AluOpType.mult)
                            nc.vector.tensor_tensor(on[:], on[:], lnb_t[:],
                                                    mybir.AluOpType.add)
                            nc.sync.dma_start(
                                out_d.ap()[tt * 128:(tt + 1) * 128, :], on[:])

    nc.finalize()
    return nc


@lru_cache(maxsize=4)
def _get_runner(n_cores, T, S, H, NH, is_pre, has_bias, repeat=1,
                kt_js=None):
    """Build + jit once; returns fn(in_maps) -> list of out dicts."""
    import jax
    import numpy as _np
    from jax.sharding import Mesh, PartitionSpec
    from jax.experimental.shard_map import shard_map
    import concourse.mybir as mybir
    from concourse import bass2jax
    from concourse.bass2jax import _bass_exec_p, install_neuronx_cc_hook

    nc = _build(n_cores, T, S, H, NH, is_pre, has_bias, repeat, kt_js=kt_js)
    install_neuronx_cc_hook()

    partition_name = (nc.partition_id_tensor.name
                      if nc.partition_id_tensor else None)
    in_names, out_names, out_avals, zero_shapes = [], [], [], []
    for alloc in nc.m.functions[0].allocations:
        if not isinstance(alloc, mybir.MemoryLocationSet):
            continue
        name = alloc.memorylocations[0].name
        if alloc.kind == "ExternalInput":
            if name != partition_name:
                in_names.append(name)
        elif alloc.kind == "ExternalOutput":
            out_names.append(name)
            shape = tuple(alloc.tensor_shape)
            dtype = mybir.dt.np(alloc.dtype)
            out_avals.append(jax.core.ShapedArray(shape, dtype))
            zero_shapes.append((shape, dtype))
    n_params = len(in_names)
    n_outs = len(out_avals)
    all_in_names = list(in_names) + list(out_names)
    if partition_name is not None:
        all_in_names.append(partition_name)

    def _body(*args):
        operands = list(args)
        if partition_name is not None:
            operands.append(bass2jax.partition_id_tensor())
        outs = _bass_exec_p.bind(
            *operands,
            out_avals=tuple(out_avals),
            in_names=tuple(all_in_names),
            out_names=tuple(out_names),
            lowering_input_output_aliases=(),
            sim_require_finite=True,
            sim_require_nnan=True,
            nc=nc,
        )
        return tuple(outs)

    devices = jax.devices()[:n_cores]
    if n_cores == 1:
        jfn = jax.jit(_body, keep_unused=True)

        def _prep(in_maps):
            args = [jax.device_put(_np.asarray(in_maps[0][n]))
                    for n in in_names]
            zeros = [jax.device_put(_np.zeros(s, d)) for s, d in zero_shapes]
            return args + zeros

        def _collect(outs):
            return [{n: _np.asarray(outs[i]) for i, n in enumerate(out_names)}]
    else:
        mesh = Mesh(np.asarray(devices), ("core",))
        from jax.sharding import NamedSharding
        shard = NamedSharding(mesh, PartitionSpec("core"))
        repl = NamedSharding(mesh, PartitionSpec())
        REPLICATED = {"qkvw", "ow", "bqkv", "lnw", "lnb", "csum"}
        in_specs = tuple(
            (PartitionSpec() if n in REPLICATED else PartitionSpec("core"))
            for n in in_names) + (PartitionSpec("core"),) * n_outs
        out_specs = (PartitionSpec("core"),) * n_outs
        jfn = jax.jit(
            shard_map(_body, mesh=mesh, in_specs=in_specs,
                      out_specs=out_specs, check_rep=False),
            keep_unused=True)

        def _prep(in_maps):
            concat_in = []
            for n in in_names:
                if n in REPLICATED:
                    concat_in.append(
                        jax.device_put(_np.asarray(in_maps[0][n]), repl))
                else:
                    concat_in.append(jax.device_put(
                        _np.concatenate([_np.asarray(m[n]) for m in in_maps],
                                        axis=0), shard))
            zeros = [
                jax.device_put(
                    _np.zeros((n_cores * s[0], *s[1:]), d), shard)
                for s, d in zero_shapes]
            return concat_in + zeros

        def _collect(outs):
            return [
                {n: _np.asarray(outs[i]).reshape(
                    n_cores, *out_avals[i].shape)[c]
                 for i, n in enumerate(out_names)}
                for c in range(n_cores)]

    class Runner:
        in_names_ = in_names
        out_names_ = out_names

        def prep(self, in_maps):
            return _prep(in_maps)

        def call(self, args):
            return jfn(*args)

        def run(self, in_maps):
            outs = jfn(*_prep(in_maps))
            jax.block_until_ready(outs)
            return _collect(outs)

        def collect(self, outs):
            return _collect(outs)

    return Runner()


def _prep_core_inputs(inp, mask, weight, bias, qkv, o, is_pre, n_cores,
                      NH=16):
    """Host-side prep: fold LN weight into qkv, pre-transpose x per core,
    build per-core input dicts."""
    import ml_dtypes
    B, S, H = inp.shape
    D = H // NH
    B_core = B // n_cores
    T = B_core * S
    KO = H // 128
    H3 = 3 * H
    KT = S // 128

    # Pre-LN: xn = z*w + b with z the normalized input, so
    # xn @ qkv = z @ (w[:,None]*qkv) + (b @ qkv): fold w into the weights
    # and b into a per-output-channel additive term applied on-device.
    # The mean subtraction uses (x-mu)@W = x@W - mu*colsum(W), with
    # colsum computed here from the bf16-rounded weights (exact algebra).
    qkvw = qkv.astype(np.float32)
    if is_pre:
        w = weight.astype(np.float32)
        if not np.all(w == 1.0):
            qkvw = qkvw * w[:, None]
        bqkv = bias.astype(np.float32) @ qkv.astype(np.float32)
    else:
        bqkv = np.zeros(H3, dtype=np.float32)
    bqkv[:H] *= np.float32(1.0 / np.sqrt(D))
    has_bias = bool(np.any(bqkv))

    qkv_b = qkvw.astype(ml_dtypes.bfloat16)
    csum = qkv_b.astype(np.float64).sum(axis=0).astype(np.float32)  # [H3]
    qkv_r = qkv_b.reshape(KO, 128, H3)
    o_r = o.astype(ml_dtypes.bfloat16).reshape(KO, 128, H)

    # Key compaction: rank batches by unmasked-key count (descending) and
    # deal them round-robin so slot j across all cores needs only
    # kt_js[j] = ceil(max_U(slot j)/128) key chunks (SPMD: one program).
    mask = np.asarray(mask)
    U = (mask == 0).sum(axis=1).astype(np.int64)  # unmasked keys per batch
    order = np.argsort(-U, kind="stable")
    KTF = S // 128
    kt_js = []
    for j in range(B_core):
        mx = int(U[order[j * n_cores:(j + 1) * n_cores]].max())
        kt_js.append(int(min(KTF, max(1, -(-mx // 128)))))
    kt_js = tuple(kt_js)
    TG = 128 * sum(kt_js)

    in_maps, bidx_all = [], []
    for c in range(n_cores):
        bidx = [int(order[j * n_cores + c]) for j in range(B_core)]
        bidx_all.append(bidx)
        xb = inp[bidx].reshape(T, H)
        xbT = np.ascontiguousarray(xb.astype(np.float32).T)  # [H, T]
        # compacted kv tokens: per slot, unmasked keys first, zero-padded
        gs = []
        for j, b in enumerate(bidx):
            idx = np.nonzero(mask[b] == 0)[0]
            g = inp[b][idx].astype(np.float32)
            pad = kt_js[j] * 128 - g.shape[0]
            if pad > 0:
                g = np.concatenate(
                    [g, np.zeros((pad, H), np.float32)], axis=0)
            gs.append(g)
        xg = np.concatenate(gs, axis=0)  # [TG, H]
        xgT = np.ascontiguousarray(xg.T)
        # additive mask over the dense per-slot key layout: real compacted
        # keys -> 0, padding and unused tail chunks -> NEG_BIG
        mb = np.zeros((B_core * KTF, 128), np.float32)
        for j, b in enumerate(bidx):
            for kt in range(KTF):
                nreal = (min(128, max(0, int(U[b]) - kt * 128))
                         if kt < kt_js[j] else 0)
                mb[j * KTF + kt, nreal:] = NEG_BIG
        m = {
            "xT": xbT.reshape(KO, 128, T).astype(ml_dtypes.bfloat16),
            "xTg": xgT.reshape(KO, 128, TG).astype(ml_dtypes.bfloat16),
            "qkvw": qkv_r,
            "ow": o_r,
            "maskb": np.ascontiguousarray(mb),
        }
        if is_pre:
            m["csum"] = np.ascontiguousarray(csum.reshape(H3 // 128, 128))
        if has_bias:
            m["bqkv"] = np.ascontiguousarray(
                bqkv.reshape(H3 // 128, 128))
        if not is_pre:
            m["lnw"] = np.ascontiguousarray(weight.astype(np.float32))
            m["lnb"] = np.ascontiguousarray(bias.astype(np.float32))
        in_maps.append(m)
    return in_maps, has_bias, (B, S, H, NH, B_core, T), {
        "kt_js": kt_js, "bidx_all": bidx_all}


def kernel(inp, mask, weight, bias, qkv, o, isPre):
    inp = np.asarray(inp)
    mask = np.asarray(mask)
    weight = np.asarray(weight)
    bias = np.asarray(bias)
    qkv = np.asarray(qkv)
    o = np.asarray(o)
    is_pre = bool(int(np.asarray(isPre)))

    n_cores = 8
    NH = 16
    in_maps, has_bias, (B, S, H, _, B_core, T), extra = _prep_core_inputs(
        inp, mask, weight, bias, qkv, o, is_pre, n_cores)

    runner = _get_runner(n_cores, T, S, H, NH, is_pre, has_bias,
                         kt_js=extra["kt_js"])
    results = runner.run(in_maps)

    out = np.empty((B, S, H), dtype=np.float32)
    for c in range(n_cores):
        if is_pre:
            outT = results[c]["outT"]  # [H, T]
            slab = outT.T.reshape(B_core, S, H)
        else:
            slab = results[c]["outN"].reshape(B_core, S, H)
        for j, b in enumerate(extra["bidx_all"][c]):
            out[b] = slab[j]
    return out


# revision 30
# speedup vs baseline: 1.0649x; 1.0101x over previous
"""Trainium2 Bass kernel for nn_MultiHeadLayer (pre-LN MHA, fused QKV).

Self-contained: takes FULL inputs, shards data-parallel over batch across
8 NeuronCores, runs a Bass/Tile kernel per core, gathers the full output.

Per-core dataflow (T = B_core*S tokens, H hidden, NH heads, D = H/NH):
  Phase 1: host supplies xT [H, T] bf16 (pre-transposed). LN without PE
           transposes: Sx = 1s @ xT and Sxx = 1s @ xT^2 ones-matmuls land
           the per-token mean/var broadcast across partitions in PSUM for
           free. The mean is NOT subtracted from x; instead the identity
           (x - mu) @ W = x @ W - mu * colsum(W) is applied at the PSUM
           evacuation of the QKV matmul together with the 1/std factor
           (colsum(W) precomputed on host). All-bf16 weight-stationary
           matmuls. q,k stream to projT [2H, T] bf16 in DRAM; v is
           PE-transposed at evacuation time into an SBUF-resident
           vn_all [tok, d] so phase 2 needs no v traffic at all.
  Phase 2: per (batch, head): scoresT = kT.T @ qT (k on partitions), exp
           fused with additive mask via per-partition ACT bias, sumexp via
           ones-matmul, ctxT from vn_all with the normalization fused into
           the PSUM evacuation. q,k are loaded per head-pair x batch-half
           group as single large row-contiguous DMAs on the sync queue;
           the scalar queue carries only exp + o-weight prefetch.
  Phase 3: outT = o.T @ ctxT (bf16), weight-stationary -> outT [H, T] ->
           host transposes during unshard.
"""

import numpy as np
from functools import lru_cache

LN_EPS = 1e-5
NEG_BIG = -1.0e30


def _build(n_cores, T, S, H, NH, is_pre, has_bias, repeat=1, kt_js=None):
    import concourse.bacc as bacc
    import concourse.mybir as mybir
    import concourse.tile as tile
    from concourse.masks import make_identity

    F32 = mybir.dt.float32
    BF16 = mybir.dt.bfloat16
    ACT = mybir.ActivationFunctionType

    KO = H // 128          # hidden-dim 128-chunks
    H3 = 3 * H
    D = H // NH
    DT = D // 128          # d-chunks per head
    KT = S // 128          # key-token 128-chunks per sequence
    B_core = T // S
    TC = T // 512          # token 512-chunks
    NCH = H3 // 128        # qkv column chunks of 128
    NQK = 2 * H // 128     # q+k column chunks (written to DRAM)

    # Key compaction: kt_js[j] = number of 128-key chunks kept for batch
    # slot j (host packs each slot's unmasked keys first, zero-padded).
    # k/v are computed over the compacted TG columns only, then scattered
    # back to the dense per-slot layout so phase 2 is compaction-agnostic.
    if kt_js is None:
        kt_js = (KT,) * B_core
    kt_js = tuple(kt_js)
    assert len(kt_js) == B_core and all(1 <= k <= KT for k in kt_js)
    NKC = sum(kt_js)
    TG = NKC * 128
    blk2dense = []
    for j in range(B_core):
        for kt in range(kt_js[j]):
            blk2dense.append(j * KT + kt)
    missing = [ch for ch in range(B_core * KT) if ch not in set(blk2dense)]
    # PSUM-sized column chunks over the compacted key tokens
    tgw, tgo, r = [], [], 0
    while r < TG:
        w = min(512, TG - r)
        tgw.append(w)
        tgo.append(r)
        r += w
    # contiguous dense-scatter runs per chunk: (src_block, dense_chunk, n)
    kruns = []
    for i, w in enumerate(tgw):
        b0 = tgo[i] // 128
        runs, s = [], 0
        while s < w // 128:
            d0 = blk2dense[b0 + s]
            n = 1
            while (s + n < w // 128
                   and blk2dense[b0 + s + n] == d0 + n):
                n += 1
            runs.append((s, d0, n))
            s += n
        kruns.append(runs)

    nc = bacc.Bacc("TRN2", target_bir_lowering=False, debug=False,
                   num_devices=n_cores)

    xT_d = nc.dram_tensor("xT", [KO, 128, T], BF16, kind="ExternalInput")
    xTg_d = nc.dram_tensor("xTg", [KO, 128, TG], BF16, kind="ExternalInput")
    qkv_d = nc.dram_tensor("qkvw", [KO, 128, H3], BF16, kind="ExternalInput")
    o_d = nc.dram_tensor("ow", [KO, 128, H], BF16, kind="ExternalInput")
    # maskb[b*KT+kt, :] = additive key-mask bias for key tokens kt*128..+128
    mb_d = nc.dram_tensor("maskb", [B_core * KT, 128], F32,
                          kind="ExternalInput")
    if is_pre:
        # csum[i, :] = sum_h qkvw_bf16[h, i*128:(i+1)*128] (host, exact)
        cs_d = nc.dram_tensor("csum", [NCH, 128], F32, kind="ExternalInput")
    if has_bias:
        # bqkv[i, :] = (bias @ qkvw)[i*128:(i+1)*128]
        bq_d = nc.dram_tensor("bqkv", [NCH, 128], F32, kind="ExternalInput")
    if is_pre:
        out_d = nc.dram_tensor("outT", [H, T], F32, kind="ExternalOutput")
    else:
        # post-LN needs LN params applied on-device to the output rows
        lnw_d = nc.dram_tensor("lnw", [H], F32, kind="ExternalInput")
        lnb_d = nc.dram_tensor("lnb", [H], F32, kind="ExternalInput")
        out_d = nc.dram_tensor("outN", [T, H], F32, kind="ExternalOutput")

    with tile.TileContext(nc) as tc:
        with tc.tile_pool(name="consts", bufs=1) as cp, \
             tc.tile_pool(name="dram", bufs=1, space="DRAM") as dp:
            ident = cp.tile([128, 128], F32)
            make_identity(nc, ident[:])
            identb = cp.tile([128, 128], BF16)
            nc.vector.tensor_copy(identb[:], ident[:])
            onesb = cp.tile([128, 128], BF16)
            nc.vector.memset(onesb[:], 1.0)
            eps_t = cp.tile([128, 1], F32)
            nc.vector.memset(eps_t[:], LN_EPS)
            mb_t = cp.tile([128, B_core * KT], F32)
            nc.sync.dma_start(mb_t[:], mb_d.ap().rearrange("i p -> p i"))
            if is_pre:
                cs_t = cp.tile([128, NCH], F32)
                nc.sync.dma_start(cs_t[:], cs_d.ap().rearrange("i p -> p i"))
            if has_bias:
                bq_t = cp.tile([128, NCH], F32)
                nc.sync.dma_start(bq_t[:], bq_d.ap().rearrange("i p -> p i"))

            qkv_ap = qkv_d.ap().rearrange("ko p n -> p ko n")
            o_ap = o_d.ap().rearrange("ko p n -> p ko n")
            projT = dp.tile([2 * H, T], BF16)
            if not is_pre:
                oTs = dp.tile([H, T], F32)
                import concourse.bass as _bass
                lnw_bc = _bass.AP(tensor=lnw_d.ap().tensor, offset=0,
                                  ap=[[0, 128], [1, H]])
                lnb_bc = _bass.AP(tensor=lnb_d.ap().tensor, offset=0,
                                  ap=[[0, 128], [1, H]])
                lnw_t = cp.tile([128, H], F32)
                nc.sync.dma_start(lnw_t[:], lnw_bc)
                lnb_t = cp.tile([128, H], F32)
                nc.sync.dma_start(lnb_t[:], lnb_bc)

            qsc = float(1.0 / np.sqrt(H // NH))

            def emit_stats(xt, w, sqp, stp, sttp, bcp, tagn, want_q):
                """ones-matmul LN stats over one rhs chunk [128, KO, w];
                returns (Rsel, NMRsel) with the q scale folded if want_q."""
                ps_mu = stp.tile([128, w], F32, tag="psmu")
                ps_v = stp.tile([128, w], F32, tag="psv")
                for ko in range(KO):
                    # square on the (idle) Vector engine: keeps the Scalar
                    # queue free to issue weight-chunk DMAs during stats.
                    xsq = sqp.tile([128, w], BF16)
                    nc.vector.tensor_tensor(xsq[:], xt[:, ko, :],
                                            xt[:, ko, :],
                                            mybir.AluOpType.mult)
                    nc.tensor.matmul(ps_mu[:], onesb[:], xt[:, ko, :],
                                     start=(ko == 0), stop=(ko == KO - 1))
                    nc.tensor.matmul(ps_v[:], onesb[:], xsq[:],
                                     start=(ko == 0), stop=(ko == KO - 1))
                negmu = sttp.tile([128, w], F32)
                nc.vector.tensor_scalar_mul(negmu[:], ps_mu[:], -1.0 / H)
                musq = sttp.tile([128, w], F32)
                nc.vector.tensor_tensor(musq[:], negmu[:], negmu[:],
                                        mybir.AluOpType.mult)
                var = sttp.tile([128, w], F32)
                nc.vector.tensor_scalar_mul(var[:], ps_v[:], 1.0 / H)
                nc.vector.tensor_tensor(var[:], var[:], musq[:],
                                        mybir.AluOpType.subtract)
                # rstd = exp(-0.5*ln(var+eps)): both on the (idle) Scalar
                # engine, no DVE reciprocal.
                lnv = sttp.tile([128, w], F32)
                nc.scalar.activation(lnv[:], var[:], ACT.Ln,
                                     bias=eps_t[:], scale=1.0)
                R = bcp.tile([128, w], F32, name=f"R{tagn}")
                nc.scalar.activation(R[:], lnv[:], ACT.Exp, scale=-0.5)
                if want_q:
                    Rq = bcp.tile([128, w], F32, name=f"Rq{tagn}")
                    nc.vector.tensor_scalar_mul(Rq[:], R[:], qsc)
                    NMRq = bcp.tile([128, w], F32, name=f"NMRq{tagn}")
                    nc.vector.tensor_tensor(NMRq[:], negmu[:], Rq[:],
                                            mybir.AluOpType.mult)
                    return Rq, NMRq
                NMR = bcp.tile([128, w], F32, name=f"NMR{tagn}")
                nc.vector.tensor_tensor(NMR[:], negmu[:], R[:],
                                        mybir.AluOpType.mult)
                return R, NMR

            def emit_evac(ev, ps, Rsel, NMRsel, nch, is_q):
                if is_pre:
                    t1 = t1p.tile(list(ev.shape), F32)
                    if has_bias:
                        nc.vector.tensor_scalar(
                            out=t1[:], in0=NMRsel[:],
                            scalar1=cs_t[:, nch:nch + 1],
                            scalar2=bq_t[:, nch:nch + 1],
                            op0=mybir.AluOpType.mult,
                            op1=mybir.AluOpType.add)
                    else:
                        nc.vector.tensor_scalar_mul(
                            t1[:], NMRsel[:], cs_t[:, nch:nch + 1])
                    tmp = tmp1.tile(list(ev.shape), F32)
                    nc.vector.tensor_tensor(tmp[:], ps[:], Rsel[:],
                                            mybir.AluOpType.mult)
                    nc.vector.tensor_tensor(ev[:], tmp[:], t1[:],
                                            mybir.AluOpType.add)
                elif is_q:
                    nc.vector.tensor_scalar_mul(ev[:], ps[:], qsc)
                else:
                    nc.vector.tensor_copy(ev[:], ps[:])

            for _rep in range(repeat):
                # vn_all[tok, dense tok-chunk, d]: v in [token, feature]
                # layout, SBUF-resident across phases 1-2.
                vstack = tc.tile_pool(name="vall", bufs=1)
                vap = vstack.__enter__()
                vn_all = vap.tile([128, T // 128, H], BF16, name="vn_all")

                # ---- Phase 1a: q projection, per token-half (fused LN) ----
                gstack = tc.tile_pool(name="xtg", bufs=1)
                gp_ = gstack.__enter__()
                xTgs = [gp_.tile([128, KO, w], BF16, name=f"xTg{i}")
                        for i, w in enumerate(tgw)]
                bgstack = tc.tile_pool(name="bcg", bufs=1)
                bcg = bgstack.__enter__()
                Rg, NMRg = [], []
                for tch in range(TC):
                    with tc.tile_pool(name="xts", bufs=1) as xp, \
                         tc.tile_pool(name="bcq", bufs=1) as bcp:
                        xt = xp.tile([128, KO, 512], BF16, name=f"xT{tch}")
                        for ko in range(KO):
                            nc.sync.dma_start(
                                xt[:, ko, :],
                                xT_d.ap()[ko, :, tch * 512:(tch + 1) * 512])
                        if tch == 0:
                            # compacted kv tokens: transfer ordered behind
                            # the first q half, well before the kv pass.
                            for i, w in enumerate(tgw):
                                for ko in range(KO):
                                    nc.sync.dma_start(
                                        xTgs[i][:, ko, :],
                                        xTg_d.ap()[ko, :,
                                                   tgo[i]:tgo[i] + w])
                        Rsel = NMRsel = None
                        if is_pre:
                            with tc.tile_pool(name="sq", bufs=3) as sqp, \
                                 tc.tile_pool(name="stps", bufs=2,
                                              space="PSUM") as stp, \
                                 tc.tile_pool(name="stt", bufs=4) as sttp:
                                Rsel, NMRsel = emit_stats(
                                    xt, 512, sqp, stp, sttp, bcp,
                                    f"q{tch}", True)
                        with tc.tile_pool(name="wch", bufs=2) as wp, \
                             tc.tile_pool(name="ev1", bufs=4) as ep, \
                             tc.tile_pool(name="t1p", bufs=2) as t1p, \
                             tc.tile_pool(name="tmp1", bufs=2) as tmp1, \
                             tc.tile_pool(name="ps1", bufs=2,
                                          space="PSUM") as pp1:
                            for nch in range(KO):
                                wt = wp.tile([128, KO, 128], BF16)
                                nc.scalar.dma_start(
                                    wt[:],
                                    qkv_ap[:, :, nch * 128:(nch + 1) * 128])
                                ps = pp1.tile([128, 512], F32)
                                for ko in range(KO):
                                    nc.tensor.matmul(
                                        ps[:], wt[:, ko], xt[:, ko, :],
                                        start=(ko == 0), stop=(ko == KO - 1))
                                ev = ep.tile([128, 512], BF16)
                                emit_evac(ev, ps, Rsel, NMRsel, nch, True)
                                nc.sync.dma_start(
                                    projT[nch * 128:(nch + 1) * 128,
                                          tch * 512:(tch + 1) * 512], ev[:])
                    if tch == 0 and is_pre:
                        # kv stats emitted between the q halves: this PE
                        # work covers the xT-t1 DMA (blocked on the xts
                        # buffer until the t0 projection finishes reading).
                        with tc.tile_pool(name="sqg", bufs=3) as sqp, \
                             tc.tile_pool(name="stpsg", bufs=2,
                                          space="PSUM") as stp, \
                             tc.tile_pool(name="sttg", bufs=4) as sttp:
                            for i, w in enumerate(tgw):
                                r_, n_ = emit_stats(xTgs[i], w, sqp, stp,
                                                    sttp, bcg, f"g{i}", False)
                                Rg.append(r_)
                                NMRg.append(n_)

                # ---- Phase 1b: k,v over compacted keys, dense scatter ----
                with tc.tile_pool(name="zp", bufs=1) as zp:
                    # zero-fill the dense k/v chunks with no compacted
                    # source: their keys are fully masked (exp -> 0), the
                    # zeros only keep the scores finite.
                    if missing:
                        zev = zp.tile([128, 128], BF16)
                        nc.vector.memset(zev[:], 0.0)
                        for nch in range(KO, NQK):
                            for ch in missing:
                                nc.sync.dma_start(
                                    projT[nch * 128:(nch + 1) * 128,
                                          ch * 128:(ch + 1) * 128], zev[:])
                        for ch in missing:
                            nc.vector.memset(vn_all[:, ch, :], 0.0)

                    with tc.tile_pool(name="wch2", bufs=2) as wp, \
                         tc.tile_pool(name="ev1g", bufs=6) as ep, \
                         tc.tile_pool(name="t1p", bufs=3) as t1p, \
                         tc.tile_pool(name="tmp1", bufs=3) as tmp1, \
                         tc.tile_pool(name="ps1g", bufs=2,
                                      space="PSUM") as pp1, \
                         tc.tile_pool(name="tps1", bufs=2,
                                      space="PSUM") as tp1:
                        pending_v = []

                        def flush_v():
                            # PE-transpose a finished v evacuation into its
                            # dense vn_all chunks (emitted one nch later so
                            # the DVE evacuation has time to complete).
                            for ev, nch, i in pending_v:
                                for s in range(tgw[i] // 128):
                                    dch = blk2dense[tgo[i] // 128 + s]
                                    pt = tp1.tile([128, 128], BF16)
                                    nc.tensor.transpose(
                                        pt[:], ev[:, s * 128:(s + 1) * 128],
                                        identb[:])
                                    nc.vector.tensor_copy(
                                        vn_all[:, dch,
                                               (nch - NQK) * 128:
                                               (nch - NQK + 1) * 128], pt[:])
                            pending_v.clear()

                        for nch in range(KO, NCH):
                            wt = wp.tile([128, KO, 128], BF16)
                            nc.scalar.dma_start(
                                wt[:], qkv_ap[:, :, nch * 128:(nch + 1) * 128])
                            for i, w in enumerate(tgw):
                                ps = pp1.tile([128, w], F32,
                                              tag=f"ps1_{i}",
                                              name=f"ps1_{i}")
                                for ko in range(KO):
                                    nc.tensor.matmul(
                                        ps[:], wt[:, ko], xTgs[i][:, ko, :],
                                        start=(ko == 0), stop=(ko == KO - 1))
                                flush_v()
                                ev = ep.tile([128, w], BF16, tag=f"ev{i}")
                                emit_evac(ev, ps,
                                          Rg[i] if is_pre else None,
                                          NMRg[i] if is_pre else None,
                                          nch, False)
                                if nch < NQK:
                                    # scatter compact blocks to their dense
                                    # column positions (contiguous runs)
                                    for s0, d0, nb in kruns[i]:
                                        nc.sync.dma_start(
                                            projT[nch * 128:(nch + 1) * 128,
                                                  d0 * 128:
                                                  (d0 + nb) * 128],
                                            ev[:, s0 * 128:(s0 + nb) * 128])
                                else:
                                    pending_v.append((ev, nch, i))
                        flush_v()
                bgstack.__exit__(None, None, None)
                gstack.__exit__(None, None, None)

                # ---------------- Phase 2: attention ----------------
                with tc.tile_pool(name="ctxt", bufs=1) as cxp:
                    # Half-token tiles: phase 3 on tokens 0-511 (batches 0-1)
                    # starts while attention runs batches 2-3.
                    ctxTs = [cxp.tile([128, KO, 512], BF16, name=f"ctxT{i}")
                             for i in range(TC)]
                    assert NH % 2 == 0 and B_core % 2 == 0
                    # phase-3 weight pool opened alongside attention: all
                    # o-weight chunks prefetch on the scalar queue (which
                    # only carries exp activations during attention).
                    p3stack = tc.tile_pool(name="och", bufs=2)
                    op_ = p3stack.__enter__()
                    ots_pre = []
                    # groups: (bh, p) = batch-half x head-pair; each group
                    # loads q,k for 2 heads x 512 tokens as one DMA each.
                    groups = [(bh, p) for bh in range(B_core // 2)
                              for p in range(NH // 2)]
                    # pairs: two per group (the two batches in the half)
                    pairs = [(g, bs) for g in range(len(groups))
                             for bs in range(2)]
                    with tc.tile_pool(name="ld2", bufs=2) as ld, \
                         tc.tile_pool(name="exp2", bufs=2) as xpp, \
                         tc.tile_pool(name="rec2", bufs=2) as rp, \
                         tc.tile_pool(name="ps2s", bufs=2, space="PSUM") as p2s, \
                         tc.tile_pool(name="ps2m", bufs=2, space="PSUM") as p2m, \
                         tc.tile_pool(name="ps2c", bufs=2, space="PSUM") as p2c:
                        gtt = {}
                        stt = {}

                        def emit_load_group(g):
                            bh, p = groups[g]
                            q_ = ld.tile([128, 2 * DT, 512], BF16, tag="qT")
                            k_ = ld.tile([128, 2 * DT, 512], BF16, tag="kT")
                            r0 = 2 * p * D
                            # first groups via the idle GPSIMD queue: the
                            # sync queue is still draining phase-1 writes
                            # when attention starts.
                            eng = nc.gpsimd if g < 2 else nc.sync
                            eng.dma_start(
                                q_[:],
                                projT[r0:r0 + 2 * D,
                                      bh * 512:(bh + 1) * 512]
                                .rearrange("(c p) t -> p c t", p=128))
                            eng.dma_start(
                                k_[:],
                                projT[H + r0:H + r0 + 2 * D,
                                      bh * 512:(bh + 1) * 512]
                                .rearrange("(c p) t -> p c t", p=128))
                            gtt[g] = dict(q=q_, k=k_)

                        def emit_produce(i):
                            g, bs = pairs[i]
                            bh, p = groups[g]
                            b = 2 * bh + bs
                            gt = gtt[g]
                            expT = xpp.tile([128, KT, 2 * S], BF16, tag="expT")
                            for kt in range(KT):
                                pss = p2s.tile([128, 2 * S], F32)
                                for h in range(2):
                                    for dt in range(DT):
                                        c = h * DT + dt
                                        nc.tensor.matmul(
                                            pss[:, h * S:(h + 1) * S],
                                            gt["k"][:, c,
                                                    bs * S + kt * 128:
                                                    bs * S + (kt + 1) * 128],
                                            gt["q"][:, c,
                                                    bs * S:(bs + 1) * S],
                                            start=(dt == 0),
                                            stop=(dt == DT - 1))
                                nc.scalar.activation(
                                    expT[:, kt], pss[:], ACT.Exp,
                                    bias=mb_t[:, b * KT + kt:b * KT + kt + 1],
                                    scale=1.0)
                            stt[i] = dict(b=b, p=p, expT=expT)

                        def emit_sumexp(i):
                            st = stt[i]
                            psm = p2m.tile([128, 2 * S], F32)
                            for kt in range(KT):
                                nc.tensor.matmul(psm[:], onesb[:],
                                                 st["expT"][:, kt],
                                                 start=(kt == 0),
                                                 stop=(kt == KT - 1))
                            # 1/sumexp on DVE: an Ln/Exp chain on the
                            # Scalar engine thrashes the activation table
                            # (1.3us reload per switch vs the scores Exp).
                            rec = rp.tile([128, 2 * S], F32, tag="rec")
                            nc.vector.reciprocal(rec[:], psm[:])
                            st["rec"] = rec

                        def emit_consume(i):
                            st = stt.pop(i)
                            b, p = st["b"], st["p"]
                            expT, rec = st["expT"], st["rec"]
                            for dt in range(DT):
                                psc = p2c.tile([128, 2 * S], F32)
                                for h in range(2):
                                    n = 2 * p + h
                                    for kt in range(KT):
                                        nc.tensor.matmul(
                                            psc[:, h * S:(h + 1) * S],
                                            vn_all[:, b * KT + kt,
                                                   n * D + dt * 128:
                                                   n * D + (dt + 1) * 128],
                                            expT[:, kt, h * S:(h + 1) * S],
                                            start=(kt == 0), stop=(kt == KT - 1))
                                for h in range(2):
                                    n = 2 * p + h
                                    nc.vector.tensor_tensor(
                                        ctxTs[b // 2][:, n * DT + dt,
                                                      (b % 2) * S:
                                                      (b % 2 + 1) * S],
                                        psc[:, h * S:(h + 1) * S],
                                        rec[:, h * S:(h + 1) * S],
                                        mybir.AluOpType.mult)

                        NPAIR = len(pairs)
                        emit_load_group(0)
                        emit_load_group(1)
                        emit_produce(0)
                        for i in range(NPAIR):
                            # sumexp first: its reciprocal runs on DVE while
                            # the PE streams the next pair's scores, so the
                            # ctx matmuls in emit_consume never wait on it.
                            emit_sumexp(i)
                            g, bs = pairs[i]
                            if bs == 0 and g + 2 < len(groups):
                                emit_load_group(g + 2)
                            if i == 0:
                                # prefetch o-weight chunks on the scalar
                                # queue (idle but for exps in phase 2).
                                for hoch in range(2):
                                    ot = op_.tile([128, KO, 128], BF16,
                                                  tag="ot")
                                    nc.gpsimd.dma_start(
                                        ot[:],
                                        o_ap[:, :,
                                             hoch * 128:(hoch + 1) * 128])
                                    ots_pre.append(ot)
                            if i + 1 < NPAIR:
                                emit_produce(i + 1)
                            emit_consume(i)

                    # ---------------- Phase 3: output projection ----------------
                    with tc.tile_pool(name="ev3", bufs=3) as e3, \
                         tc.tile_pool(name="ps3", bufs=2, space="PSUM") as pp3:
                        for hoch in range(KO):
                            if hoch < 2:
                                ot = ots_pre[hoch]
                            else:
                                ot = op_.tile([128, KO, 128], BF16, tag="ot")
                                nc.gpsimd.dma_start(
                                    ot[:],
                                    o_ap[:, :, hoch * 128:(hoch + 1) * 128])
                            psl = [pp3.tile([128, 512], F32, tag=f"ps3_{t}",
                                            name=f"ps3_{t}")
                                   for t in range(TC)]
                            for tch in range(TC):
                                for ko in range(KO):
                                    nc.tensor.matmul(
                                        psl[tch][:], ot[:, ko],
                                        ctxTs[tch][:, ko, :],
                                        start=(ko == 0), stop=(ko == KO - 1))
                            for tch in range(TC):
                                ps = psl[tch]
                                ev = e3.tile([128, 512], F32)
                                nc.vector.tensor_copy(ev[:], ps[:])
                                dst = (out_d.ap() if is_pre else oTs)
                                nc.sync.dma_start(
                                    dst[hoch * 128:(hoch + 1) * 128,
                                        tch * 512:(tch + 1) * 512], ev[:])
                    p3stack.__exit__(None, None, None)
                vstack.__exit__(None, None, None)

                # ---------------- Phase 4 (isPre=0): transpose + post-LN -------
                if not is_pre:
                    with tc.tile_pool(name="p4in", bufs=3) as p4i, \
                         tc.tile_pool(name="p4out", bufs=2) as p4o, \
                         tc.tile_pool(name="st4", bufs=8) as st4, \
                         tc.tile_pool(name="sq4", bufs=2) as sq4, \
                         tc.tile_pool(name="tps4", bufs=4, space="PSUM") as tp4:
                        for tt in range(T // 128):
                            on = p4o.tile([128, H], F32)
                            for hh in range(KO):
                                it = p4i.tile([128, 128], F32)
                                nc.sync.dma_start(
                                    it[:], oTs[hh * 128:(hh + 1) * 128,
                                               tt * 128:(tt + 1) * 128])
                                pt = tp4.tile([128, 128], F32)
                                nc.tensor.transpose(pt[:], it[:], ident[:])
                                nc.vector.tensor_copy(
                                    on[:, hh * 128:(hh + 1) * 128], pt[:])
                            ssum = st4.tile([128, 1], F32)
                            nc.vector.reduce_sum(out=ssum[:], in_=on[:],
                                                 axis=mybir.AxisListType.X)
                            negmu = st4.tile([128, 1], F32)
                            nc.vector.tensor_scalar_mul(negmu[:], ssum[:], -1.0 / H)
                            xsq = sq4.tile([128, H], F32)
                            vsum = st4.tile([128, 1], F32)
                            nc.scalar.activation(xsq[:], on[:], ACT.Square,
                                                 bias=negmu[:], scale=1.0,
                                                 accum_out=vsum[:])
                            sd = st4.tile([128, 1], F32)
                            nc.scalar.activation(sd[:], vsum[:], ACT.Sqrt,
                                                 bias=eps_t[:], scale=1.0 / H)
                            rstd = st4.tile([128, 1], F32)
                            nc.vector.reciprocal(rstd[:], sd[:])
                            nc.vector.tensor_scalar(
                                out=on[:], in0=on[:],
                                scalar1=negmu[:], scalar2=rstd[:],
                                op0=mybir.AluOpType.add,
                                op1=mybir.AluOpType.mult)
                            nc.vector.tensor_tensor(on[:], on[:], lnw_t[:],
                                                    mybir.AluOpType.mult)
                            nc.vector.tensor_tensor(on[:], on[:], lnb_t[:],
                                                    mybir.AluOpType.add)
                            nc.sync.dma_start(
                                out_d.ap()[tt * 128:(tt + 1) * 128, :], on[:])

    nc.finalize()
    return nc


@lru_cache(maxsize=4)
def _get_runner(n_cores, T, S, H, NH, is_pre, has_bias, repeat=1,
                kt_js=None):
    """Build + jit once; returns fn(in_maps) -> list of out dicts."""
    import jax
    import numpy as _np
    from jax.sharding import Mesh, PartitionSpec
    from jax.experimental.shard_map import shard_map
    import concourse.mybir as mybir
    from concourse import bass2jax
    from concourse.bass2jax import _bass_exec_p, install_neuronx_cc_hook

    nc = _build(n_cores, T, S, H, NH, is_pre, has_bias, repeat, kt_js=kt_js)
    install_neuronx_cc_hook()

    partition_name = (nc.partition_id_tensor.name
                      if nc.partition_id_tensor else None)
    in_names, out_names, out_avals, zero_shapes = [], [], [], []
    for alloc in nc.m.functions[0].allocations:
        if not isinstance(alloc, mybir.MemoryLocationSet):
            continue
        name = alloc.memorylocations[0].name
        if alloc.kind == "ExternalInput":
            if name != partition_name:
                in_names.append(name)
        elif alloc.kind == "ExternalOutput":
            out_names.append(name)
            shape = tuple(alloc.tensor_shape)
            dtype = mybir.dt.np(alloc.dtype)
            out_avals.append(jax.core.ShapedArray(shape, dtype))
            zero_shapes.append((shape, dtype))
    n_params = len(in_names)
    n_outs = len(out_avals)
    all_in_names = list(in_names) + list(out_names)
    if partition_name is not None:
        all_in_names.append(partition_name)

    def _body(*args):
        operands = list(args)
        if partition_name is not None:
            operands.append(bass2jax.partition_id_tensor())
        outs = _bass_exec_p.bind(
            *operands,
            out_avals=tuple(out_avals),
            in_names=tuple(all_in_names),
            out_names=tuple(out_names),
            lowering_input_output_aliases=(),
            sim_require_finite=True,
            sim_require_nnan=True,
            nc=nc,
        )
        return tuple(outs)

    devices = jax.devices()[:n_cores]
    if n_cores == 1:
        jfn = jax.jit(_body, keep_unused=True)

        def _prep(in_maps):
            args = [jax.device_put(_np.asarray(in_maps[0][n]))
                    for n in in_names]
            zeros = [jax.device_put(_np.zeros(s, d)) for s, d in zero_shapes]
            return args + zeros

        def _collect(outs):
            return [{n: _np.asarray(outs[i]) for i, n in enumerate(out_names)}]
    else:
        mesh = Mesh(np.asarray(devices), ("core",))
        from jax.sharding import NamedSharding
        shard = NamedSharding(mesh, PartitionSpec("core"))
        repl = NamedSharding(mesh, PartitionSpec())
        REPLICATED = {"qkvw", "ow", "bqkv", "lnw", "lnb", "csum"}
        in_specs = tuple(
            (PartitionSpec() if n in REPLICATED else PartitionSpec("core"))
            for n in in_names) + (PartitionSpec("core"),) * n_outs
        out_specs = (PartitionSpec("core"),) * n_outs
        jfn = jax.jit(
            shard_map(_body, mesh=mesh, in_specs=in_specs,
                      out_specs=out_specs, check_rep=False),
            keep_unused=True)

        def _prep(in_maps):
            concat_in = []
            for n in in_names:
                if n in REPLICATED:
                    concat_in.append(
                        jax.device_put(_np.asarray(in_maps[0][n]), repl))
                else:
                    concat_in.append(jax.device_put(
                        _np.concatenate([_np.asarray(m[n]) for m in in_maps],
                                        axis=0), shard))
            zeros = [
                jax.device_put(
                    _np.zeros((n_cores * s[0], *s[1:]), d), shard)
                for s, d in zero_shapes]
            return concat_in + zeros

        def _collect(outs):
            return [
                {n: _np.asarray(outs[i]).reshape(
                    n_cores, *out_avals[i].shape)[c]
                 for i, n in enumerate(out_names)}
                for c in range(n_cores)]

    class Runner:
        in_names_ = in_names
        out_names_ = out_names

        def prep(self, in_maps):
            return _prep(in_maps)

        def call(self, args):
            return jfn(*args)

        def run(self, in_maps):
            outs = jfn(*_prep(in_maps))
            jax.block_until_ready(outs)
            return _collect(outs)

        def collect(self, outs):
            return _collect(outs)

    return Runner()


def _prep_core_inputs(inp, mask, weight, bias, qkv, o, is_pre, n_cores,
                      NH=16):
    """Host-side prep: fold LN weight into qkv, pre-transpose x per core,
    build per-core input dicts."""
    import ml_dtypes
    B, S, H = inp.shape
    D = H // NH
    B_core = B // n_cores
    T = B_core * S
    KO = H // 128
    H3 = 3 * H
    KT = S // 128

    # Pre-LN: xn = z*w + b with z the normalized input, so
    # xn @ qkv = z @ (w[:,None]*qkv) + (b @ qkv): fold w into the weights
    # and b into a per-output-channel additive term applied on-device.
    # The mean subtraction uses (x-mu)@W = x@W - mu*colsum(W), with
    # colsum computed here from the bf16-rounded weights (exact algebra).
    qkvw = qkv.astype(np.float32)
    if is_pre:
        w = weight.astype(np.float32)
        if not np.all(w == 1.0):
            qkvw = qkvw * w[:, None]
        bqkv = bias.astype(np.float32) @ qkv.astype(np.float32)
    else:
        bqkv = np.zeros(H3, dtype=np.float32)
    bqkv[:H] *= np.float32(1.0 / np.sqrt(D))
    has_bias = bool(np.any(bqkv))

    qkv_b = qkvw.astype(ml_dtypes.bfloat16)
    csum = qkv_b.astype(np.float64).sum(axis=0).astype(np.float32)  # [H3]
    qkv_r = qkv_b.reshape(KO, 128, H3)
    o_r = o.astype(ml_dtypes.bfloat16).reshape(KO, 128, H)

    # Key compaction: rank batches by unmasked-key count (descending) and
    # deal them round-robin so slot j across all cores needs only
    # kt_js[j] = ceil(max_U(slot j)/128) key chunks (SPMD: one program).
    mask = np.asarray(mask)
    U = (mask == 0).sum(axis=1).astype(np.int64)  # unmasked keys per batch
    order = np.argsort(-U, kind="stable")
    KTF = S // 128
    kt_js = []
    for j in range(B_core):
        mx = int(U[order[j * n_cores:(j + 1) * n_cores]].max())
        kt_js.append(int(min(KTF, max(1, -(-mx // 128)))))
    kt_js = tuple(kt_js)
    TG = 128 * sum(kt_js)

    in_maps, bidx_all = [], []
    for c in range(n_cores):
        bidx = [int(order[j * n_cores + c]) for j in range(B_core)]
        bidx_all.append(bidx)
        xb = inp[bidx].reshape(T, H)
        xbT = np.ascontiguousarray(xb.astype(np.float32).T)  # [H, T]
        # compacted kv tokens: per slot, unmasked keys first, zero-padded
        gs = []
        for j, b in enumerate(bidx):
            idx = np.nonzero(mask[b] == 0)[0]
            g = inp[b][idx].astype(np.float32)
            pad = kt_js[j] * 128 - g.shape[0]
            if pad > 0:
                g = np.concatenate(
                    [g, np.zeros((pad, H), np.float32)], axis=0)
            gs.append(g)
        xg = np.concatenate(gs, axis=0)  # [TG, H]
        xgT = np.ascontiguousarray(xg.T)
        # additive mask over the dense per-slot key layout: real compacted
        # keys -> 0, padding and unused tail chunks -> NEG_BIG
        mb = np.zeros((B_core * KTF, 128), np.float32)
        for j, b in enumerate(bidx):
            for kt in range(KTF):
                nreal = (min(128, max(0, int(U[b]) - kt * 128))
                         if kt < kt_js[j] else 0)
                mb[j * KTF + kt, nreal:] = NEG_BIG
        m = {
            "xT": xbT.reshape(KO, 128, T).astype(ml_dtypes.bfloat16),
            "xTg": xgT.reshape(KO, 128, TG).astype(ml_dtypes.bfloat16),
            "qkvw": qkv_r,
            "ow": o_r,
            "maskb": np.ascontiguousarray(mb),
        }
        if is_pre:
            m["csum"] = np.ascontiguousarray(csum.reshape(H3 // 128, 128))
        if has_bias:
            m["bqkv"] = np.ascontiguousarray(
                bqkv.reshape(H3 // 128, 128))
        if not is_pre:
            m["lnw"] = np.ascontiguousarray(weight.astype(np.float32))
            m["lnb"] = np.ascontiguousarray(bias.astype(np.float32))
        in_maps.append(m)
    return in_maps, has_bias, (B, S, H, NH, B_core, T), {
        "kt_js": kt_js, "bidx_all": bidx_all}


def kernel(inp, mask, weight, bias, qkv, o, isPre):
    inp = np.asarray(inp)
    mask = np.asarray(mask)
    weight = np.asarray(weight)
    bias = np.asarray(bias)
    qkv = np.asarray(qkv)
    o = np.asarray(o)
    is_pre = bool(int(np.asarray(isPre)))

    n_cores = 8
    NH = 16
    in_maps, has_bias, (B, S, H, _, B_core, T), extra = _prep_core_inputs(
        inp, mask, weight, bias, qkv, o, is_pre, n_cores)

    runner = _get_runner(n_cores, T, S, H, NH, is_pre, has_bias,
                         kt_js=extra["kt_js"])
    results = runner.run(in_maps)

    out = np.empty((B, S, H), dtype=np.float32)
    for c in range(n_cores):
        if is_pre:
            outT = results[c]["outT"]  # [H, T]
            slab = outT.T.reshape(B_core, S, H)
        else:
            slab = results[c]["outN"].reshape(B_core, S, H)
        for j, b in enumerate(extra["bidx_all"][c]):
            out[b] = slab[j]
    return out


# revision 34
# speedup vs baseline: 1.0681x; 1.0029x over previous
"""Trainium2 Bass kernel for nn_MultiHeadLayer (pre-LN MHA, fused QKV).

Self-contained: takes FULL inputs, shards data-parallel over batch across
8 NeuronCores, runs a Bass/Tile kernel per core, gathers the full output.

Per-core dataflow (T = B_core*S tokens, H hidden, NH heads, D = H/NH):
  Phase 1: host supplies xT [H, T] bf16 (pre-transposed). LN without PE
           transposes: Sx = 1s @ xT and Sxx = 1s @ xT^2 ones-matmuls land
           the per-token mean/var broadcast across partitions in PSUM for
           free. The mean is NOT subtracted from x; instead the identity
           (x - mu) @ W = x @ W - mu * colsum(W) is applied at the PSUM
           evacuation of the QKV matmul together with the 1/std factor
           (colsum(W) precomputed on host). All-bf16 weight-stationary
           matmuls. q,k stream to projT [2H, T] bf16 in DRAM; v is
           PE-transposed at evacuation time into an SBUF-resident
           vn_all [tok, d] so phase 2 needs no v traffic at all.
  Phase 2: per (batch, head): scoresT = kT.T @ qT (k on partitions), exp
           fused with additive mask via per-partition ACT bias, sumexp via
           ones-matmul, ctxT from vn_all with the normalization fused into
           the PSUM evacuation. q,k are loaded per head-pair x batch-half
           group as single large row-contiguous DMAs on the sync queue;
           the scalar queue carries only exp + o-weight prefetch.
  Phase 3: outT = o.T @ ctxT (bf16), weight-stationary -> outT [H, T] ->
           host transposes during unshard.
"""

import numpy as np
from functools import lru_cache

LN_EPS = 1e-5
NEG_BIG = -1.0e30


def _build(n_cores, T, S, H, NH, is_pre, has_bias, repeat=1, kt_js=None):
    import concourse.bacc as bacc
    import concourse.mybir as mybir
    import concourse.tile as tile
    from concourse.masks import make_identity

    F32 = mybir.dt.float32
    BF16 = mybir.dt.bfloat16
    ACT = mybir.ActivationFunctionType

    KO = H // 128          # hidden-dim 128-chunks
    H3 = 3 * H
    D = H // NH
    DT = D // 128          # d-chunks per head
    KT = S // 128          # key-token 128-chunks per sequence
    B_core = T // S
    TC = T // 512          # token 512-chunks
    NCH = H3 // 128        # qkv column chunks of 128
    NQK = 2 * H // 128     # q+k column chunks (written to DRAM)

    # Key compaction: kt_js[j] = number of 128-key chunks kept for batch
    # slot j (host packs each slot's unmasked keys first, zero-padded).
    # k/v are computed over the compacted TG columns only, then scattered
    # back to the dense per-slot layout so phase 2 is compaction-agnostic.
    if kt_js is None:
        kt_js = (KT,) * B_core
    kt_js = tuple(kt_js)
    assert len(kt_js) == B_core and all(1 <= k <= KT for k in kt_js)
    NKC = sum(kt_js)
    TG = NKC * 128
    blk2dense = []
    for j in range(B_core):
        for kt in range(kt_js[j]):
            blk2dense.append(j * KT + kt)
    missing = [ch for ch in range(B_core * KT) if ch not in set(blk2dense)]
    # PSUM-sized column chunks over the compacted key tokens
    tgw, tgo, r = [], [], 0
    while r < TG:
        w = min(512, TG - r)
        tgw.append(w)
        tgo.append(r)
        r += w
    # contiguous dense-scatter runs per chunk: (src_block, dense_chunk, n)
    kruns = []
    for i, w in enumerate(tgw):
        b0 = tgo[i] // 128
        runs, s = [], 0
        while s < w // 128:
            d0 = blk2dense[b0 + s]
            n = 1
            while (s + n < w // 128
                   and blk2dense[b0 + s + n] == d0 + n):
                n += 1
            runs.append((s, d0, n))
            s += n
        kruns.append(runs)

    nc = bacc.Bacc("TRN2", target_bir_lowering=False, debug=False,
                   num_devices=n_cores)

    xT_d = nc.dram_tensor("xT", [KO, 128, T], BF16, kind="ExternalInput")
    xTg_d = nc.dram_tensor("xTg", [KO, 128, TG], BF16, kind="ExternalInput")
    qkv_d = nc.dram_tensor("qkvw", [KO, 128, H3], BF16, kind="ExternalInput")
    o_d = nc.dram_tensor("ow", [KO, 128, H], BF16, kind="ExternalInput")
    # maskb[b*KT+kt, :] = additive key-mask bias for key tokens kt*128..+128
    mb_d = nc.dram_tensor("maskb", [B_core * KT, 128], F32,
                          kind="ExternalInput")
    if is_pre:
        # csum[i, :] = sum_h qkvw_bf16[h, i*128:(i+1)*128] (host, exact)
        cs_d = nc.dram_tensor("csum", [NCH, 128], F32, kind="ExternalInput")
    if has_bias:
        # bqkv[i, :] = (bias @ qkvw)[i*128:(i+1)*128]
        bq_d = nc.dram_tensor("bqkv", [NCH, 128], F32, kind="ExternalInput")
    if is_pre:
        out_d = nc.dram_tensor("outT", [H, T], F32, kind="ExternalOutput")
    else:
        # post-LN needs LN params applied on-device to the output rows
        lnw_d = nc.dram_tensor("lnw", [H], F32, kind="ExternalInput")
        lnb_d = nc.dram_tensor("lnb", [H], F32, kind="ExternalInput")
        out_d = nc.dram_tensor("outN", [T, H], F32, kind="ExternalOutput")

    with tile.TileContext(nc) as tc:
        with tc.tile_pool(name="consts", bufs=1) as cp, \
             tc.tile_pool(name="dram", bufs=1, space="DRAM") as dp:
            ident = cp.tile([128, 128], F32)
            make_identity(nc, ident[:])
            identb = cp.tile([128, 128], BF16)
            nc.vector.tensor_copy(identb[:], ident[:])
            onesb = cp.tile([128, 128], BF16)
            nc.vector.memset(onesb[:], 1.0)
            eps_t = cp.tile([128, 1], F32)
            nc.vector.memset(eps_t[:], LN_EPS)
            mb_t = cp.tile([128, B_core * KT], F32)
            nc.sync.dma_start(mb_t[:], mb_d.ap().rearrange("i p -> p i"))
            if is_pre:
                cs_t = cp.tile([128, NCH], F32)
                nc.sync.dma_start(cs_t[:], cs_d.ap().rearrange("i p -> p i"))
            if has_bias:
                bq_t = cp.tile([128, NCH], F32)
                nc.sync.dma_start(bq_t[:], bq_d.ap().rearrange("i p -> p i"))

            qkv_ap = qkv_d.ap().rearrange("ko p n -> p ko n")
            o_ap = o_d.ap().rearrange("ko p n -> p ko n")
            projT = dp.tile([2 * H, T], BF16)
            if not is_pre:
                oTs = dp.tile([H, T], F32)
                import concourse.bass as _bass
                lnw_bc = _bass.AP(tensor=lnw_d.ap().tensor, offset=0,
                                  ap=[[0, 128], [1, H]])
                lnb_bc = _bass.AP(tensor=lnb_d.ap().tensor, offset=0,
                                  ap=[[0, 128], [1, H]])
                lnw_t = cp.tile([128, H], F32)
                nc.sync.dma_start(lnw_t[:], lnw_bc)
                lnb_t = cp.tile([128, H], F32)
                nc.sync.dma_start(lnb_t[:], lnb_bc)

            qsc = float(1.0 / np.sqrt(H // NH))

            def emit_stats(xt, w, sqp, stp, sttp, bcp, tagn, want_q):
                """ones-matmul LN stats over one rhs chunk [128, KO, w];
                returns (Rsel, NMRsel) with the q scale folded if want_q."""
                ps_mu = stp.tile([128, w], F32, tag="psmu")
                ps_v = stp.tile([128, w], F32, tag="psv")
                for ko in range(KO):
                    # square on the (idle) Vector engine: keeps the Scalar
                    # queue free to issue weight-chunk DMAs during stats.
                    xsq = sqp.tile([128, w], BF16)
                    nc.vector.tensor_tensor(xsq[:], xt[:, ko, :],
                                            xt[:, ko, :],
                                            mybir.AluOpType.mult)
                    nc.tensor.matmul(ps_mu[:], onesb[:], xt[:, ko, :],
                                     start=(ko == 0), stop=(ko == KO - 1))
                    nc.tensor.matmul(ps_v[:], onesb[:], xsq[:],
                                     start=(ko == 0), stop=(ko == KO - 1))
                negmu = sttp.tile([128, w], F32)
                nc.vector.tensor_scalar_mul(negmu[:], ps_mu[:], -1.0 / H)
                musq = sttp.tile([128, w], F32)
                nc.vector.tensor_tensor(musq[:], negmu[:], negmu[:],
                                        mybir.AluOpType.mult)
                var = sttp.tile([128, w], F32)
                nc.vector.tensor_scalar_mul(var[:], ps_v[:], 1.0 / H)
                nc.vector.tensor_tensor(var[:], var[:], musq[:],
                                        mybir.AluOpType.subtract)
                # rstd = exp(-0.5*ln(var+eps)): both on the (idle) Scalar
                # engine, no DVE reciprocal.
                lnv = sttp.tile([128, w], F32)
                nc.scalar.activation(lnv[:], var[:], ACT.Ln,
                                     bias=eps_t[:], scale=1.0)
                R = bcp.tile([128, w], F32, name=f"R{tagn}")
                nc.scalar.activation(R[:], lnv[:], ACT.Exp, scale=-0.5)
                if want_q:
                    Rq = bcp.tile([128, w], F32, name=f"Rq{tagn}")
                    nc.vector.tensor_scalar_mul(Rq[:], R[:], qsc)
                    NMRq = bcp.tile([128, w], F32, name=f"NMRq{tagn}")
                    nc.vector.tensor_tensor(NMRq[:], negmu[:], Rq[:],
                                            mybir.AluOpType.mult)
                    return Rq, NMRq
                NMR = bcp.tile([128, w], F32, name=f"NMR{tagn}")
                nc.vector.tensor_tensor(NMR[:], negmu[:], R[:],
                                        mybir.AluOpType.mult)
                return R, NMR

            def emit_evac(ev, ps, Rsel, NMRsel, nch, is_q):
                if is_pre:
                    t1 = t1p.tile(list(ev.shape), F32)
                    if has_bias:
                        nc.vector.tensor_scalar(
                            out=t1[:], in0=NMRsel[:],
                            scalar1=cs_t[:, nch:nch + 1],
                            scalar2=bq_t[:, nch:nch + 1],
                            op0=mybir.AluOpType.mult,
                            op1=mybir.AluOpType.add)
                    else:
                        nc.vector.tensor_scalar_mul(
                            t1[:], NMRsel[:], cs_t[:, nch:nch + 1])
                    tmp = tmp1.tile(list(ev.shape), F32)
                    nc.vector.tensor_tensor(tmp[:], ps[:], Rsel[:],
                                            mybir.AluOpType.mult)
                    nc.vector.tensor_tensor(ev[:], tmp[:], t1[:],
                                            mybir.AluOpType.add)
                elif is_q:
                    nc.vector.tensor_scalar_mul(ev[:], ps[:], qsc)
                else:
                    nc.vector.tensor_copy(ev[:], ps[:])

            for _rep in range(repeat):
                # vn_all[tok, dense tok-chunk, d]: v in [token, feature]
                # layout, SBUF-resident across phases 1-2.
                vstack = tc.tile_pool(name="vall", bufs=1)
                vap = vstack.__enter__()
                vn_all = vap.tile([128, T // 128, H], BF16, name="vn_all")

                # ---- Phase 1a: q projection, per token-half (fused LN) ----
                gstack = tc.tile_pool(name="xtg", bufs=1)
                gp_ = gstack.__enter__()
                xTgs = [gp_.tile([128, KO, w], BF16, name=f"xTg{i}")
                        for i, w in enumerate(tgw)]
                bgstack = tc.tile_pool(name="bcg", bufs=1)
                bcg = bgstack.__enter__()
                Rg, NMRg = [], []
                for tch in range(TC):
                    with tc.tile_pool(name="xts", bufs=1) as xp, \
                         tc.tile_pool(name="bcq", bufs=1) as bcp:
                        xt = xp.tile([128, KO, 512], BF16, name=f"xT{tch}")
                        for ko in range(KO):
                            nc.sync.dma_start(
                                xt[:, ko, :],
                                xT_d.ap()[ko, :, tch * 512:(tch + 1) * 512])
                        if tch == 0:
                            # compacted kv tokens: transfer ordered behind
                            # the first q half, well before the kv pass.
                            for i, w in enumerate(tgw):
                                for ko in range(KO):
                                    nc.sync.dma_start(
                                        xTgs[i][:, ko, :],
                                        xTg_d.ap()[ko, :,
                                                   tgo[i]:tgo[i] + w])
                        Rsel = NMRsel = None
                        if is_pre:
                            with tc.tile_pool(name="sq", bufs=3) as sqp, \
                                 tc.tile_pool(name="stps", bufs=2,
                                              space="PSUM") as stp, \
                                 tc.tile_pool(name="stt", bufs=4) as sttp:
                                Rsel, NMRsel = emit_stats(
                                    xt, 512, sqp, stp, sttp, bcp,
                                    f"q{tch}", True)
                        with tc.tile_pool(name="wch", bufs=2) as wp, \
                             tc.tile_pool(name="ev1", bufs=4) as ep, \
                             tc.tile_pool(name="t1p", bufs=2) as t1p, \
                             tc.tile_pool(name="tmp1", bufs=2) as tmp1, \
                             tc.tile_pool(name="ps1", bufs=2,
                                          space="PSUM") as pp1:
                            for nch in range(KO):
                                wt = wp.tile([128, KO, 128], BF16)
                                nc.scalar.dma_start(
                                    wt[:],
                                    qkv_ap[:, :, nch * 128:(nch + 1) * 128])
                                ps = pp1.tile([128, 512], F32)
                                for ko in range(KO):
                                    nc.tensor.matmul(
                                        ps[:], wt[:, ko], xt[:, ko, :],
                                        start=(ko == 0), stop=(ko == KO - 1))
                                ev = ep.tile([128, 512], BF16)
                                emit_evac(ev, ps, Rsel, NMRsel, nch, True)
                                nc.sync.dma_start(
                                    projT[nch * 128:(nch + 1) * 128,
                                          tch * 512:(tch + 1) * 512], ev[:])
                    if tch == 0 and is_pre:
                        # kv stats emitted between the q halves: this PE
                        # work covers the xT-t1 DMA (blocked on the xts
                        # buffer until the t0 projection finishes reading).
                        with tc.tile_pool(name="sqg", bufs=3) as sqp, \
                             tc.tile_pool(name="stpsg", bufs=2,
                                          space="PSUM") as stp, \
                             tc.tile_pool(name="sttg", bufs=4) as sttp:
                            for i, w in enumerate(tgw):
                                r_, n_ = emit_stats(xTgs[i], w, sqp, stp,
                                                    sttp, bcg, f"g{i}", False)
                                Rg.append(r_)
                                NMRg.append(n_)

                # ---- Phase 1b: k,v over compacted keys, dense scatter ----
                with tc.tile_pool(name="zp", bufs=1) as zp:
                    # zero-fill the dense k/v chunks with no compacted
                    # source: their keys are fully masked (exp -> 0), the
                    # zeros only keep the scores finite.
                    if missing:
                        zev = zp.tile([128, 128], BF16)
                        nc.vector.memset(zev[:], 0.0)
                        for nch in range(KO, NQK):
                            for ch in missing:
                                nc.sync.dma_start(
                                    projT[nch * 128:(nch + 1) * 128,
                                          ch * 128:(ch + 1) * 128], zev[:])
                        for ch in missing:
                            nc.vector.memset(vn_all[:, ch, :], 0.0)

                    with tc.tile_pool(name="wch2", bufs=2) as wp, \
                         tc.tile_pool(name="ev1g", bufs=6) as ep, \
                         tc.tile_pool(name="t1p", bufs=3) as t1p, \
                         tc.tile_pool(name="tmp1", bufs=3) as tmp1, \
                         tc.tile_pool(name="ps1g", bufs=2,
                                      space="PSUM") as pp1, \
                         tc.tile_pool(name="tps1", bufs=2,
                                      space="PSUM") as tp1:
                        pending_v = []

                        def flush_v():
                            # PE-transpose a finished v evacuation into its
                            # dense vn_all chunks (emitted one nch later so
                            # the DVE evacuation has time to complete).
                            for ev, nch, i in pending_v:
                                for s in range(tgw[i] // 128):
                                    dch = blk2dense[tgo[i] // 128 + s]
                                    pt = tp1.tile([128, 128], BF16)
                                    nc.tensor.transpose(
                                        pt[:], ev[:, s * 128:(s + 1) * 128],
                                        identb[:])
                                    nc.vector.tensor_copy(
                                        vn_all[:, dch,
                                               (nch - NQK) * 128:
                                               (nch - NQK + 1) * 128], pt[:])
                            pending_v.clear()

                        for nch in range(KO, NCH):
                            wt = wp.tile([128, KO, 128], BF16)
                            nc.scalar.dma_start(
                                wt[:], qkv_ap[:, :, nch * 128:(nch + 1) * 128])
                            for i, w in enumerate(tgw):
                                ps = pp1.tile([128, w], F32,
                                              tag=f"ps1_{i}",
                                              name=f"ps1_{i}")
                                for ko in range(KO):
                                    nc.tensor.matmul(
                                        ps[:], wt[:, ko], xTgs[i][:, ko, :],
                                        start=(ko == 0), stop=(ko == KO - 1))
                                flush_v()
                                ev = ep.tile([128, w], BF16, tag=f"ev{i}")
                                emit_evac(ev, ps,
                                          Rg[i] if is_pre else None,
                                          NMRg[i] if is_pre else None,
                                          nch, False)
                                if nch < NQK:
                                    # scatter compact blocks to their dense
                                    # column positions (contiguous runs)
                                    for s0, d0, nb in kruns[i]:
                                        nc.sync.dma_start(
                                            projT[nch * 128:(nch + 1) * 128,
                                                  d0 * 128:
                                                  (d0 + nb) * 128],
                                            ev[:, s0 * 128:(s0 + nb) * 128])
                                else:
                                    pending_v.append((ev, nch, i))
                        flush_v()
                bgstack.__exit__(None, None, None)
                gstack.__exit__(None, None, None)

                # ---------------- Phase 2: attention ----------------
                with tc.tile_pool(name="ctxt", bufs=1) as cxp:
                    # Half-token tiles: phase 3 on tokens 0-511 (batches 0-1)
                    # starts while attention runs batches 2-3.
                    ctxTs = [cxp.tile([128, KO, 512], BF16, name=f"ctxT{i}")
                             for i in range(TC)]
                    assert NH % 2 == 0 and B_core % 2 == 0
                    # phase-3 weight pool opened alongside attention: all
                    # o-weight chunks prefetch on the scalar queue (which
                    # only carries exp activations during attention).
                    p3stack = tc.tile_pool(name="och", bufs=2)
                    op_ = p3stack.__enter__()
                    ots_pre = []
                    # groups: (bh, p) = batch-half x head-pair; each group
                    # loads q,k for 2 heads x 512 tokens as one DMA each.
                    groups = [(bh, p) for bh in range(B_core // 2)
                              for p in range(NH // 2)]
                    # pairs: two per group (the two batches in the half)
                    pairs = [(g, bs) for g in range(len(groups))
                             for bs in range(2)]
                    with tc.tile_pool(name="ld2", bufs=2) as ld, \
                         tc.tile_pool(name="exp2", bufs=2) as xpp, \
                         tc.tile_pool(name="rec2", bufs=2) as rp, \
                         tc.tile_pool(name="ps2s", bufs=2, space="PSUM") as p2s, \
                         tc.tile_pool(name="ps2m", bufs=2, space="PSUM") as p2m, \
                         tc.tile_pool(name="ps2c", bufs=2, space="PSUM") as p2c:
                        gtt = {}
                        stt = {}

                        def emit_load_group(g):
                            bh, p = groups[g]
                            q_ = ld.tile([128, 2 * DT, 512], BF16, tag="qT")
                            k_ = ld.tile([128, 2 * DT, 512], BF16, tag="kT")
                            r0 = 2 * p * D
                            # first groups via the idle GPSIMD queue: the
                            # sync queue is still draining phase-1 writes
                            # when attention starts.
                            eng = nc.gpsimd if g < 2 else nc.sync
                            eng.dma_start(
                                q_[:],
                                projT[r0:r0 + 2 * D,
                                      bh * 512:(bh + 1) * 512]
                                .rearrange("(c p) t -> p c t", p=128))
                            eng.dma_start(
                                k_[:],
                                projT[H + r0:H + r0 + 2 * D,
                                      bh * 512:(bh + 1) * 512]
                                .rearrange("(c p) t -> p c t", p=128))
                            gtt[g] = dict(q=q_, k=k_)

                        def emit_produce(i):
                            g, bs = pairs[i]
                            bh, p = groups[g]
                            b = 2 * bh + bs
                            gt = gtt[g]
                            expT = xpp.tile([128, KT, 2 * S], BF16, tag="expT")
                            for kt in range(KT):
                                pss = p2s.tile([128, 2 * S], F32)
                                for h in range(2):
                                    for dt in range(DT):
                                        c = h * DT + dt
                                        nc.tensor.matmul(
                                            pss[:, h * S:(h + 1) * S],
                                            gt["k"][:, c,
                                                    bs * S + kt * 128:
                                                    bs * S + (kt + 1) * 128],
                                            gt["q"][:, c,
                                                    bs * S:(bs + 1) * S],
                                            start=(dt == 0),
                                            stop=(dt == DT - 1))
                                nc.scalar.activation(
                                    expT[:, kt], pss[:], ACT.Exp,
                                    bias=mb_t[:, b * KT + kt:b * KT + kt + 1],
                                    scale=1.0)
                            stt[i] = dict(b=b, p=p, expT=expT)

                        def emit_sumexp(i):
                            st = stt[i]
                            psm = p2m.tile([128, 2 * S], F32)
                            for kt in range(KT):
                                nc.tensor.matmul(psm[:], onesb[:],
                                                 st["expT"][:, kt],
                                                 start=(kt == 0),
                                                 stop=(kt == KT - 1))
                            # 1/sumexp on DVE: an Ln/Exp chain on the
                            # Scalar engine thrashes the activation table
                            # (1.3us reload per switch vs the scores Exp).
                            rec = rp.tile([128, 2 * S], F32, tag="rec")
                            nc.vector.reciprocal(rec[:], psm[:])
                            st["rec"] = rec

                        def emit_consume(i):
                            st = stt.pop(i)
                            b, p = st["b"], st["p"]
                            expT, rec = st["expT"], st["rec"]
                            for dt in range(DT):
                                psc = p2c.tile([128, 2 * S], F32)
                                for h in range(2):
                                    n = 2 * p + h
                                    for kt in range(KT):
                                        nc.tensor.matmul(
                                            psc[:, h * S:(h + 1) * S],
                                            vn_all[:, b * KT + kt,
                                                   n * D + dt * 128:
                                                   n * D + (dt + 1) * 128],
                                            expT[:, kt, h * S:(h + 1) * S],
                                            start=(kt == 0), stop=(kt == KT - 1))
                                for h in range(2):
                                    n = 2 * p + h
                                    nc.vector.tensor_tensor(
                                        ctxTs[b // 2][:, n * DT + dt,
                                                      (b % 2) * S:
                                                      (b % 2 + 1) * S],
                                        psc[:, h * S:(h + 1) * S],
                                        rec[:, h * S:(h + 1) * S],
                                        mybir.AluOpType.mult)

                        NPAIR = len(pairs)
                        emit_load_group(0)
                        emit_load_group(1)
                        emit_produce(0)
                        for i in range(NPAIR):
                            # sumexp first: its reciprocal runs on DVE while
                            # the PE streams the next pair's scores, so the
                            # ctx matmuls in emit_consume never wait on it.
                            emit_sumexp(i)
                            g, bs = pairs[i]
                            if bs == 0 and g + 2 < len(groups):
                                emit_load_group(g + 2)
                            if i == 0:
                                # prefetch o-weight chunks on the scalar
                                # queue (idle but for exps in phase 2).
                                for hoch in range(2):
                                    ot = op_.tile([128, KO, 128], BF16,
                                                  tag="ot")
                                    nc.gpsimd.dma_start(
                                        ot[:],
                                        o_ap[:, :,
                                             hoch * 128:(hoch + 1) * 128])
                                    ots_pre.append(ot)
                            if i + 1 < NPAIR:
                                emit_produce(i + 1)
                            emit_consume(i)

                    # ---------------- Phase 3: output projection ----------------
                    with tc.tile_pool(name="ev3", bufs=3) as e3, \
                         tc.tile_pool(name="ps3", bufs=2, space="PSUM") as pp3:
                        for hoch in range(KO):
                            if hoch < 2:
                                ot = ots_pre[hoch]
                            else:
                                ot = op_.tile([128, KO, 128], BF16, tag="ot")
                                nc.gpsimd.dma_start(
                                    ot[:],
                                    o_ap[:, :, hoch * 128:(hoch + 1) * 128])
                            psl = [pp3.tile([128, 512], F32, tag=f"ps3_{t}",
                                            name=f"ps3_{t}")
                                   for t in range(TC)]
                            for tch in range(TC):
                                for ko in range(KO):
                                    nc.tensor.matmul(
                                        psl[tch][:], ot[:, ko],
                                        ctxTs[tch][:, ko, :],
                                        start=(ko == 0), stop=(ko == KO - 1))
                            for tch in range(TC):
                                ps = psl[tch]
                                ev = e3.tile([128, 512], F32)
                                nc.vector.tensor_copy(ev[:], ps[:])
                                dst = (out_d.ap() if is_pre else oTs)
                                nc.sync.dma_start(
                                    dst[hoch * 128:(hoch + 1) * 128,
                                        tch * 512:(tch + 1) * 512], ev[:])
                    p3stack.__exit__(None, None, None)
                vstack.__exit__(None, None, None)

                # ---------------- Phase 4 (isPre=0): transpose + post-LN -------
                if not is_pre:
                    with tc.tile_pool(name="p4in", bufs=3) as p4i, \
                         tc.tile_pool(name="p4out", bufs=2) as p4o, \
                         tc.tile_pool(name="st4", bufs=8) as st4, \
                         tc.tile_pool(name="sq4", bufs=2) as sq4, \
                         tc.tile_pool(name="tps4", bufs=4, space="PSUM") as tp4:
                        for tt in range(T // 128):
                            on = p4o.tile([128, H], F32)
                            for hh in range(KO):
                                it = p4i.tile([128, 128], F32)
                                nc.sync.dma_start(
                                    it[:], oTs[hh * 128:(hh + 1) * 128,
                                               tt * 128:(tt + 1) * 128])
                                pt = tp4.tile([128, 128], F32)
                                nc.tensor.transpose(pt[:], it[:], ident[:])
                                nc.vector.tensor_copy(
                                    on[:, hh * 128:(hh + 1) * 128], pt[:])
                            ssum = st4.tile([128, 1], F32)
                            nc.vector.reduce_sum(out=ssum[:], in_=on[:],
                                                 axis=mybir.AxisListType.X)
                            negmu = st4.tile([128, 1], F32)
                            nc.vector.tensor_scalar_mul(negmu[:], ssum[:], -1.0 / H)
                            xsq = sq4.tile([128, H], F32)
                            vsum = st4.tile([128, 1], F32)
                            nc.scalar.activation(xsq[:], on[:], ACT.Square,
                                                 bias=negmu[:], scale=1.0,
                                                 accum_out=vsum[:])
                            sd = st4.tile([128, 1], F32)
                            nc.scalar.activation(sd[:], vsum[:], ACT.Sqrt,
                                                 bias=eps_t[:], scale=1.0 / H)
                            rstd = st4.tile([128, 1], F32)
                            nc.vector.reciprocal(rstd[:], sd[:])
                            nc.vector.tensor_scalar(
                                out=on[:], in0=on[:],
                                scalar1=negmu[:], scalar2=rstd[:],
                                op0=mybir.AluOpType.add,
                                op1=mybir.AluOpType.mult)
                            nc.vector.tensor_tensor(on[:], on[:], lnw_t[:],
                                                    mybir.AluOpType.mult)
                            nc.vector.tensor_tensor(on[:], on[:], lnb_t[:],
                                                    mybir.AluOpType.add)
                            nc.sync.dma_start(
                                out_d.ap()[tt * 128:(tt + 1) * 128, :], on[:])

    nc.finalize()
    return nc


@lru_cache(maxsize=4)
def _get_runner(n_cores, T, S, H, NH, is_pre, has_bias, repeat=1,
                kt_js=None):
    """Build + jit once; returns fn(in_maps) -> list of out dicts."""
    import jax
    import numpy as _np
    from jax.sharding import Mesh, PartitionSpec
    from jax.experimental.shard_map import shard_map
    import concourse.mybir as mybir
    from concourse import bass2jax
    from concourse.bass2jax import _bass_exec_p, install_neuronx_cc_hook

    nc = _build(n_cores, T, S, H, NH, is_pre, has_bias, repeat, kt_js=kt_js)
    install_neuronx_cc_hook()

    partition_name = (nc.partition_id_tensor.name
                      if nc.partition_id_tensor else None)
    in_names, out_names, out_avals, zero_shapes = [], [], [], []
    for alloc in nc.m.functions[0].allocations:
        if not isinstance(alloc, mybir.MemoryLocationSet):
            continue
        name = alloc.memorylocations[0].name
        if alloc.kind == "ExternalInput":
            if name != partition_name:
                in_names.append(name)
        elif alloc.kind == "ExternalOutput":
            out_names.append(name)
            shape = tuple(alloc.tensor_shape)
            dtype = mybir.dt.np(alloc.dtype)
            out_avals.append(jax.core.ShapedArray(shape, dtype))
            zero_shapes.append((shape, dtype))
    n_params = len(in_names)
    n_outs = len(out_avals)
    all_in_names = list(in_names) + list(out_names)
    if partition_name is not None:
        all_in_names.append(partition_name)

    def _body(*args):
        operands = list(args)
        if partition_name is not None:
            operands.append(bass2jax.partition_id_tensor())
        outs = _bass_exec_p.bind(
            *operands,
            out_avals=tuple(out_avals),
            in_names=tuple(all_in_names),
            out_names=tuple(out_names),
            lowering_input_output_aliases=(),
            sim_require_finite=True,
            sim_require_nnan=True,
            nc=nc,
        )
        return tuple(outs)

    devices = jax.devices()[:n_cores]
    if n_cores == 1:
        jfn = jax.jit(_body, keep_unused=True)

        def _prep(in_maps):
            args = [jax.device_put(_np.asarray(in_maps[0][n]))
                    for n in in_names]
            zeros = [jax.device_put(_np.zeros(s, d)) for s, d in zero_shapes]
            return args + zeros

        def _collect(outs):
            return [{n: _np.asarray(outs[i]) for i, n in enumerate(out_names)}]
    else:
        mesh = Mesh(np.asarray(devices), ("core",))
        from jax.sharding import NamedSharding
        shard = NamedSharding(mesh, PartitionSpec("core"))
        repl = NamedSharding(mesh, PartitionSpec())
        REPLICATED = {"qkvw", "ow", "bqkv", "lnw", "lnb", "csum"}
        in_specs = tuple(
            (PartitionSpec() if n in REPLICATED else PartitionSpec("core"))
            for n in in_names) + (PartitionSpec("core"),) * n_outs
        out_specs = (PartitionSpec("core"),) * n_outs
        jfn = jax.jit(
            shard_map(_body, mesh=mesh, in_specs=in_specs,
                      out_specs=out_specs, check_rep=False),
            keep_unused=True)

        def _prep(in_maps):
            concat_in = []
            for n in in_names:
                if n in REPLICATED:
                    concat_in.append(
                        jax.device_put(_np.asarray(in_maps[0][n]), repl))
                else:
                    concat_in.append(jax.device_put(
                        _np.concatenate([_np.asarray(m[n]) for m in in_maps],
                                        axis=0), shard))
            zeros = [
                jax.device_put(
                    _np.zeros((n_cores * s[0], *s[1:]), d), shard)
                for s, d in zero_shapes]
            return concat_in + zeros

        def _collect(outs):
            return [
                {n: _np.asarray(outs[i]).reshape(
                    n_cores, *out_avals[i].shape)[c]
                 for i, n in enumerate(out_names)}
                for c in range(n_cores)]

    class Runner:
        in_names_ = in_names
        out_names_ = out_names

        def prep(self, in_maps):
            return _prep(in_maps)

        def call(self, args):
            return jfn(*args)

        def run(self, in_maps):
            outs = jfn(*_prep(in_maps))
            jax.block_until_ready(outs)
            return _collect(outs)

        def collect(self, outs):
            return _collect(outs)

    return Runner()


def _prep_core_inputs(inp, mask, weight, bias, qkv, o, is_pre, n_cores,
                      NH=16):
    """Host-side prep: fold LN weight into qkv, pre-transpose x per core,
    build per-core input dicts."""
    import ml_dtypes
    B, S, H = inp.shape
    D = H // NH
    B_core = B // n_cores
    T = B_core * S
    KO = H // 128
    H3 = 3 * H
    KT = S // 128

    # Pre-LN: xn = z*w + b with z the normalized input, so
    # xn @ qkv = z @ (w[:,None]*qkv) + (b @ qkv): fold w into the weights
    # and b into a per-output-channel additive term applied on-device.
    # The mean subtraction uses (x-mu)@W = x@W - mu*colsum(W), with
    # colsum computed here from the bf16-rounded weights (exact algebra).
    qkvw = qkv.astype(np.float32)
    if is_pre:
        w = weight.astype(np.float32)
        if not np.all(w == 1.0):
            qkvw = qkvw * w[:, None]
        bqkv = bias.astype(np.float32) @ qkv.astype(np.float32)
    else:
        bqkv = np.zeros(H3, dtype=np.float32)
    bqkv[:H] *= np.float32(1.0 / np.sqrt(D))
    has_bias = bool(np.any(bqkv))

    qkv_b = qkvw.astype(ml_dtypes.bfloat16)
    csum = qkv_b.astype(np.float64).sum(axis=0).astype(np.float32)  # [H3]
    qkv_r = qkv_b.reshape(KO, 128, H3)
    o_r = o.astype(ml_dtypes.bfloat16).reshape(KO, 128, H)

    # Key compaction: rank batches by unmasked-key count (descending) and
    # deal them round-robin so slot j across all cores needs only
    # kt_js[j] = ceil(max_U(slot j)/128) key chunks (SPMD: one program).
    mask = np.asarray(mask)
    U = (mask == 0).sum(axis=1).astype(np.int64)  # unmasked keys per batch
    order = np.argsort(-U, kind="stable")
    KTF = S // 128
    kt_js = []
    for j in range(B_core):
        mx = int(U[order[j * n_cores:(j + 1) * n_cores]].max())
        kt_js.append(int(min(KTF, max(1, -(-mx // 128)))))
    kt_js = tuple(kt_js)
    TG = 128 * sum(kt_js)

    in_maps, bidx_all = [], []
    for c in range(n_cores):
        bidx = [int(order[j * n_cores + c]) for j in range(B_core)]
        bidx_all.append(bidx)
        xb = inp[bidx].reshape(T, H)
        xbT = np.ascontiguousarray(xb.astype(np.float32).T)  # [H, T]
        # compacted kv tokens: per slot, unmasked keys first, zero-padded
        gs = []
        for j, b in enumerate(bidx):
            idx = np.nonzero(mask[b] == 0)[0]
            g = inp[b][idx].astype(np.float32)
            pad = kt_js[j] * 128 - g.shape[0]
            if pad > 0:
                g = np.concatenate(
                    [g, np.zeros((pad, H), np.float32)], axis=0)
            gs.append(g)
        xg = np.concatenate(gs, axis=0)  # [TG, H]
        xgT = np.ascontiguousarray(xg.T)
        # additive mask over the dense per-slot key layout: real compacted
        # keys -> 0, padding and unused tail chunks -> NEG_BIG
        mb = np.zeros((B_core * KTF, 128), np.float32)
        for j, b in enumerate(bidx):
            for kt in range(KTF):
                nreal = (min(128, max(0, int(U[b]) - kt * 128))
                         if kt < kt_js[j] else 0)
                mb[j * KTF + kt, nreal:] = NEG_BIG
        m = {
            "xT": xbT.reshape(KO, 128, T).astype(ml_dtypes.bfloat16),
            "xTg": xgT.reshape(KO, 128, TG).astype(ml_dtypes.bfloat16),
            "qkvw": qkv_r,
            "ow": o_r,
            "maskb": np.ascontiguousarray(mb),
        }
        if is_pre:
            m["csum"] = np.ascontiguousarray(csum.reshape(H3 // 128, 128))
        if has_bias:
            m["bqkv"] = np.ascontiguousarray(
                bqkv.reshape(H3 // 128, 128))
        if not is_pre:
            m["lnw"] = np.ascontiguousarray(weight.astype(np.float32))
            m["lnb"] = np.ascontiguousarray(bias.astype(np.float32))
        in_maps.append(m)
    return in_maps, has_bias, (B, S, H, NH, B_core, T), {
        "kt_js": kt_js, "bidx_all": bidx_all}


def kernel(inp, mask, weight, bias, qkv, o, isPre):
    inp = np.asarray(inp)
    mask = np.asarray(mask)
    weight = np.asarray(weight)
    bias = np.asarray(bias)
    qkv = np.asarray(qkv)
    o = np.asarray(o)
    is_pre = bool(int(np.asarray(isPre)))

    n_cores = 8
    NH = 16
    in_maps, has_bias, (B, S, H, _, B_core, T), extra = _prep_core_inputs(
        inp, mask, weight, bias, qkv, o, is_pre, n_cores)

    runner = _get_runner(n_cores, T, S, H, NH, is_pre, has_bias,
                         kt_js=extra["kt_js"])
    results = runner.run(in_maps)

    out = np.empty((B, S, H), dtype=np.float32)
    for c in range(n_cores):
        if is_pre:
            outT = results[c]["outT"]  # [H, T]
            slab = outT.T.reshape(B_core, S, H)
        else:
            slab = results[c]["outN"].reshape(B_core, S, H)
        for j, b in enumerate(extra["bidx_all"][c]):
            out[b] = slab[j]
    return out
